# revision 1
# baseline (speedup 1.0000x reference)
"""ContextualAttention TRN2 kernel.

Full inputs -> full output. Sharding: 8 cores = 2 batches x 4 q-blocks of the
L=4096 attention-column dimension. Each core computes, for its 1024 columns q:

  S[p, q]  = sum_f wn[f, p] * pr[f, q]          (QK^T, K = 1152 = 9 x 128)
  E[p, q]  = exp(10 * (S - denom_q))             (denom_q = per-column softmax
                                                  shift; upper-bounds the column
                                                  max by Cauchy-Schwarz, so the
                                                  softmax is exact after the
                                                  1/colsum normalization)
  A[p, q]  = E * mfilt_p                         (post-softmax patch mask)
  colT[q,:] = (A^T @ xu) / colsum_q              (conv_transpose as GEMM)

Host side: unfold / normalization prep (pure index shuffles + one divide) and
the final col2im overlap-add.  wn has the pre-softmax mask and 1/denom_p
folded in on the host.
"""
import numpy as np

import concourse.bass as bass
import concourse.bacc as bacc
import concourse.mybir as mybir
from concourse import tile
from concourse.bass_utils import run_bass_kernel_spmd

F32 = mybir.dt.float32
F32R = mybir.dt.float32r   # full-rate (1 cyc/row, N>=256) reduced-mult fp32
AFT = mybir.ActivationFunctionType

B, C, H, W = 2, 128, 128, 128
RATE, BS = 2, 3                # attention rate, block size
Hr, Wr = H // RATE, W // RATE  # 64, 64
L = Hr * Wr                    # 4096
F = C * BS * BS                # 1152 contraction dim, 9 k-tiles
CK = C * 16                    # 2048 deconv output cols (kappa*128 + c)
QPC = L // 4                   # 1024 q columns per core
EPS = 1e-4
SCALE = 10.0
N_CORES = 8

_CACHE = {}


def _build_nc():
    nc = bacc.Bacc(None)
    wn_d = nc.declare_dram_parameter("wn", [F, L], F32R, isOutput=False)
    prq_d = nc.declare_dram_parameter("prq", [F, QPC], F32R, isOutput=False)
    xu_d = nc.declare_dram_parameter("xu", [L, CK], F32R, isOutput=False)
    ndq_d = nc.declare_dram_parameter("ndq", [1, QPC], F32R, isOutput=False)
    ones_d = nc.declare_dram_parameter("ones1", [1, 128], F32R, isOutput=False)
    mrow_d = nc.declare_dram_parameter("mrow", [128, 32], F32, isOutput=False)
    col_d = nc.declare_dram_parameter("col", [QPC, CK], F32, isOutput=True)

    NPT = L // 128    # 32 p tiles
    NKT = F // 128    # 9 k tiles
    NQT = QPC // 128  # 8 q tiles
    NCH = CK // 512   # 4 ck chunks

    with tile.TileContext(nc) as tc:
        with (
            tc.tile_pool(name="apool", bufs=NPT) as apool,
            tc.tile_pool(name="const", bufs=1) as cpool,
            tc.tile_pool(name="rhs", bufs=1) as rhspool,
            tc.tile_pool(name="lhs", bufs=2) as lhspool,
            tc.tile_pool(name="xus", bufs=3) as xupool,
            tc.tile_pool(name="outs", bufs=2) as opool,
            tc.tile_pool(name="rows", bufs=2) as rowpool,
            tc.tile_pool(name="ps", bufs=8, space="PSUM") as pspool,
        ):
            # ---- resident loads -------------------------------------------
            rhs_sb = rhspool.tile([128, NKT * QPC], F32R)       # 36 KB/part
            nc.sync.dma_start(
                rhs_sb[:].rearrange("p (k q) -> p k q", k=NKT),
                prq_d[:].rearrange("(k fi) q -> fi k q", fi=128))
            ndq_sb = rowpool.tile([1, QPC], F32R, tag="row")
            nc.sync.dma_start(ndq_sb[:], ndq_d[:])
            m_sb = cpool.tile([128, 32], F32)
            nc.sync.dma_start(m_sb[:], mrow_d[:])
            onek1 = cpool.tile([1, 128], F32R)
            nc.sync.dma_start(onek1[:], ones_d[:])
            ones_col = cpool.tile([128, 1], F32)
            nc.gpsimd.memset(ones_col[:], 1.0)
            acc = cpool.tile([128, QPC], F32)
            nc.gpsimd.memset(acc[:], 0.0)
            r8 = cpool.tile([128, NQT], F32)

            # ---- phase A: S = wn^T @ prq, E = exp(10(S-dq)), acc += E -----
            a_tiles = []
            for pt in range(NPT):
                lhs = lhspool.tile([128, NKT * 128], F32R)
                (nc.gpsimd if pt % 2 else nc.sync).dma_start(
                    lhs[:].rearrange("p (k j) -> p k j", k=NKT),
                    wn_d[:, pt * 128:(pt + 1) * 128]
                    .rearrange("(k fi) j -> fi k j", fi=128))
                at = apool.tile([128, QPC], F32R)
                for qc in range(QPC // 512):
                    ps = pspool.tile([128, 512], F32, tag="ps")
                    nc.tensor.matmul(
                        ps[:], onek1[:],
                        ndq_sb[0:1, qc * 512:(qc + 1) * 512],
                        start=True, stop=False)
                    for k in range(NKT):
                        nc.tensor.matmul(
                            ps[:],
                            lhs[:, k * 128:(k + 1) * 128],
                            rhs_sb[:, k * QPC + qc * 512:
                                   k * QPC + qc * 512 + 512],
                            start=False, stop=(k == NKT - 1))
                    nc.scalar.activation(
                        at[:, qc * 512:(qc + 1) * 512], ps[:], AFT.Exp,
                        bias=m_sb[:, pt:pt + 1], scale=SCALE)
                nc.vector.tensor_add(acc[:], acc[:], at[:].bitcast(F32))
                a_tiles.append(at)

            # ---- phase B: colsum -> r8[i, qt] = 1/colsum(q=qt*128+i) ------
            # out[m, 0] = sum_k acc[k, qt*128+m]: per-partition layout direct
            for qt in range(NQT):
                cs_ps = pspool.tile([128, 1], F32, tag="ps", name=f"csps{qt}")
                nc.tensor.matmul(
                    cs_ps[:], acc[:, qt * 128:(qt + 1) * 128], ones_col[:],
                    start=True, stop=True)
                nc.vector.tensor_copy(r8[:, qt:qt + 1], cs_ps[:])
            nc.vector.reciprocal(r8[:], r8[:])

            # ---- phase C: colT[q, ck] = sum_p A[p, q] xu[p, ck], scaled ---
            for ch in range(NCH):
                ps_c = [pspool.tile([128, 512], F32, tag="ps",
                                    name=f"psc{ch}_{i}")
                        for i in range(NQT)]
                for pt in range(NPT):
                    xt = xupool.tile([128, 512], F32R)
                    (nc.gpsimd if pt % 2 else nc.sync).dma_start(
                        xt[:], xu_d[pt * 128:(pt + 1) * 128,
                                    ch * 512:(ch + 1) * 512])
                    for qt in range(NQT):
                        nc.tensor.matmul(
                            ps_c[qt][:],
                            a_tiles[pt][:, qt * 128:(qt + 1) * 128],
                            xt[:],
                            start=(pt == 0), stop=(pt == NPT - 1))
                for qt in range(NQT):
                    ot = opool.tile([128, 512], F32)
                    nc.vector.tensor_scalar_mul(ot[:], ps_c[qt][:],
                                                r8[:, qt:qt + 1])
                    nc.scalar.dma_start(
                        col_d[qt * 128:(qt + 1) * 128,
                              ch * 512:(ch + 1) * 512], ot[:])
    nc.compile()
    return nc


def _host_prep(x, mask):
    """Per-batch GEMM-ready operands (kappa-major feature layout)."""
    out = []
    for b in range(B):
        xr = x[b, :, ::RATE, ::RATE]
        xrp = np.pad(xr, ((0, 0), (1, 1), (1, 1)))
        pr = np.empty((9, C, L), np.float32)
        for di in range(3):
            for dj in range(3):
                pr[di * 3 + dj] = xrp[:, di:di + Hr, dj:dj + Wr].reshape(C, L)
        pr = pr.reshape(F, L)
        denom = np.sqrt((pr * pr).sum(0, dtype=np.float64).astype(np.float32)
                        + np.float32(F * EPS))

        mr = mask[b, :, ::RATE, ::RATE]
        mrp = np.pad(mr, ((0, 0), (1, 1), (1, 1)))
        msum = np.zeros((1, L), np.float32)
        for di in range(3):
            for dj in range(3):
                msum += mrp[:, di:di + Hr, dj:dj + Wr].reshape(1, L)
        mfilt = (msum[0] == 0.0).astype(np.float32)

        wn = (pr / denom[None, :]) * mfilt[None, :]

        xp = np.pad(x[b], ((0, 0), (1, 1), (1, 1)))
        xu = np.empty((L, 16, C), np.float32)
        for i in range(4):
            for j in range(4):
                blk = xp[:, i:i + 2 * Hr:2, j:j + 2 * Wr:2]
                xu[:, i * 4 + j, :] = blk.reshape(C, L).T
        out.append((np.ascontiguousarray(wn), pr, denom, mfilt,
                    np.ascontiguousarray(xu.reshape(L, CK))))
    return out


def _col2im(col):
    """col [L, CK] -> [C, H, W] overlap-add, /4."""
    canvas = np.zeros((C, H + 2, W + 2), np.float32)
    blk = col.reshape(Hr, Wr, 16, C)
    for i in range(4):
        for j in range(4):
            canvas[:, i:i + 2 * Hr:2, j:j + 2 * Wr:2] += \
                blk[:, :, i * 4 + j, :].transpose(2, 0, 1)
    return canvas[:, 1:1 + H, 1:1 + W] / 4.0


def kernel(x, mask):
    x = np.asarray(x, np.float32)
    mask = np.asarray(mask, np.float32)
    if "nc" not in _CACHE:
        _CACHE["nc"] = _build_nc()
    nc = _CACHE["nc"]

    prep = _host_prep(x, mask)
    in_maps = []
    for core in range(N_CORES):
        b, g = divmod(core, 4)
        wn, pr, denom, mfilt, xu = prep[b]
        q0 = g * QPC
        in_maps.append({
            "wn": wn,
            "prq": np.ascontiguousarray(pr[:, q0:q0 + QPC]),
            "xu": xu,
            "ndq": np.ascontiguousarray(-denom[None, q0:q0 + QPC]),
            "mrow": np.ascontiguousarray(((mfilt - 1.0) * 1e4).reshape(32, 128).T),
            "ones1": np.ones((1, 128), np.float32),
        })

    _CACHE["in_maps"] = in_maps
    res = run_bass_kernel_spmd(nc, in_maps, list(range(N_CORES)))

    out = np.empty((B, C, H, W), np.float32)
    for b in range(B):
        col = np.concatenate(
            [res.results[b * 4 + g]["col"] for g in range(4)], axis=0)
        out[b] = _col2im(col)
    return out



# revision 2
# speedup vs baseline: 1.1230x; 1.1230x over previous
"""ContextualAttention TRN2 kernel.

Full inputs -> full output. Sharding: 8 cores = 2 batches x 4 q-blocks of the
L=4096 attention-column dimension. Each core computes, for its 1024 columns q:

  S[p, q]  = sum_f wn[f, p] * pr[f, q]          (QK^T, K = 1152 = 9 x 128)
  E[p, q]  = exp(10 * S - 10*c + m_p)           (c = global shift; any per-q
                                                 factor cancels in the colsum
                                                 normalization, so a single
                                                 constant that keeps exp in
                                                 f32 range is exact)
  A[p, q]  = E * mfilt_p                         (post-softmax patch mask)
  colT[q,:] = (A^T @ xu) / colsum_q              (conv_transpose as GEMM)

Host side: unfold / normalization prep (pure index shuffles + one divide) and
the final col2im overlap-add.  wn has the pre-softmax mask and 1/denom_p
folded in on the host.  All GEMM operands travel as bf16.
"""
import numpy as np
import ml_dtypes

import concourse.bass as bass
import concourse.bacc as bacc
import concourse.mybir as mybir
from concourse import tile
from concourse.bass_utils import run_bass_kernel_spmd

F32 = mybir.dt.float32
BF16 = mybir.dt.bfloat16
AFT = mybir.ActivationFunctionType

B, C, H, W = 2, 128, 128, 128
RATE, BS = 2, 3                # attention rate, block size
Hr, Wr = H // RATE, W // RATE  # 64, 64
L = Hr * Wr                    # 4096
F = C * BS * BS                # 1152 contraction dim, 9 k-tiles
CK = C * 16                    # 2048 deconv output cols (kappa*128 + c)
QPC = L // 4                   # 1024 q columns per core
EPS = 1e-4
SCALE = 10.0
N_CORES = 8
NPBF16 = ml_dtypes.bfloat16

_CACHE = {}


def _build_nc():
    nc = bacc.Bacc(None)
    wn_d = nc.declare_dram_parameter("wn", [F, L], BF16, isOutput=False)
    prq_d = nc.declare_dram_parameter("prq", [F, QPC], BF16, isOutput=False)
    xu_d = nc.declare_dram_parameter("xu", [L, CK], BF16, isOutput=False)
    mrow_d = nc.declare_dram_parameter("mrow", [128, 32], F32, isOutput=False)
    col_d = nc.declare_dram_parameter("col", [QPC, CK], F32, isOutput=True)

    NPT = L // 128    # 32 p tiles
    NKT = F // 128    # 9 k tiles
    NQT = QPC // 128  # 8 q tiles
    NCH = CK // 512   # 4 ck chunks

    with tile.TileContext(nc) as tc:
        with (
            tc.tile_pool(name="apool", bufs=NPT) as apool,
            tc.tile_pool(name="const", bufs=1) as cpool,
            tc.tile_pool(name="rhs", bufs=1) as rhspool,
            tc.tile_pool(name="lhs", bufs=2) as lhspool,
            tc.tile_pool(name="xus", bufs=3) as xupool,
            tc.tile_pool(name="outs", bufs=2) as opool,
            tc.tile_pool(name="rows", bufs=2) as rowpool,
            tc.tile_pool(name="ps", bufs=8, space="PSUM") as pspool,
        ):
            # ---- resident loads -------------------------------------------
            # per-k chunks so the first matmul only waits for its own slice
            rhs_sb = rhspool.tile([128, NKT * QPC], BF16)       # 18 KB/part
            rhs_r = rhs_sb[:].rearrange("p (k q) -> p k q", k=NKT)
            prq_r = prq_d[:].rearrange("(k fi) q -> fi k q", fi=128)
            for k in range(NKT):
                (nc.sync if k % 2 else nc.gpsimd).dma_start(
                    rhs_r[:, k], prq_r[:, k])
            m_sb = cpool.tile([128, 32], F32)
            nc.sync.dma_start(m_sb[:], mrow_d[:])
            ones_col = cpool.tile([128, 1], F32)
            nc.gpsimd.memset(ones_col[:], 1.0)
            acc = cpool.tile([128, QPC], F32)
            nc.gpsimd.memset(acc[:], 0.0)
            r8 = cpool.tile([128, NQT], F32)

            # ---- phase A: S = wn^T @ prq, E = exp(10 S + bias), acc += E --
            a_tiles = []
            for pt in range(NPT):
                lhs = lhspool.tile([128, NKT * 128], BF16)
                (nc.gpsimd if pt % 2 else nc.sync).dma_start(
                    lhs[:].rearrange("p (k j) -> p k j", k=NKT),
                    wn_d[:, pt * 128:(pt + 1) * 128]
                    .rearrange("(k fi) j -> fi k j", fi=128))
                at = apool.tile([128, QPC], BF16)
                for qc in range(QPC // 512):
                    ps = pspool.tile([128, 512], F32, tag="ps")
                    for k in range(NKT):
                        nc.tensor.matmul(
                            ps[:],
                            lhs[:, k * 128:(k + 1) * 128],
                            rhs_sb[:, k * QPC + qc * 512:
                                   k * QPC + qc * 512 + 512],
                            start=(k == 0), stop=(k == NKT - 1))
                    nc.scalar.activation(
                        at[:, qc * 512:(qc + 1) * 512], ps[:], AFT.Exp,
                        bias=m_sb[:, pt:pt + 1], scale=SCALE)
                nc.vector.tensor_add(acc[:], acc[:], at[:])
                a_tiles.append(at)

            # ---- phase B: colsum -> r8[i, qt] = 1/colsum(q=qt*128+i) ------
            # out[m, 0] = sum_k acc[k, qt*128+m]: per-partition layout direct
            for qt in range(NQT):
                cs_ps = pspool.tile([128, 1], F32, tag="ps", name=f"csps{qt}")
                nc.tensor.matmul(
                    cs_ps[:], acc[:, qt * 128:(qt + 1) * 128], ones_col[:],
                    start=True, stop=True)
                nc.vector.tensor_copy(r8[:, qt:qt + 1], cs_ps[:])
            nc.vector.reciprocal(r8[:], r8[:])

            # ---- phase C: colT[q, ck] = sum_p A[p, q] xu[p, ck], scaled ---
            for ch in range(NCH):
                ps_c = [pspool.tile([128, 512], F32, tag="ps",
                                    name=f"psc{ch}_{i}")
                        for i in range(NQT)]
                for pt in range(NPT):
                    xt = xupool.tile([128, 512], BF16)
                    (nc.gpsimd if pt % 2 else nc.sync).dma_start(
                        xt[:], xu_d[pt * 128:(pt + 1) * 128,
                                    ch * 512:(ch + 1) * 512])
                    for qt in range(NQT):
                        nc.tensor.matmul(
                            ps_c[qt][:],
                            a_tiles[pt][:, qt * 128:(qt + 1) * 128],
                            xt[:],
                            start=(pt == 0), stop=(pt == NPT - 1))
                for qt in range(NQT):
                    ot = opool.tile([128, 512], F32)
                    nc.vector.tensor_scalar_mul(ot[:], ps_c[qt][:],
                                                r8[:, qt:qt + 1])
                    nc.scalar.dma_start(
                        col_d[qt * 128:(qt + 1) * 128,
                              ch * 512:(ch + 1) * 512], ot[:])
    nc.compile()
    return nc


def _host_prep(x, mask):
    """Per-batch GEMM-ready operands (kappa-major feature layout)."""
    out = []
    for b in range(B):
        xr = x[b, :, ::RATE, ::RATE]
        xrp = np.pad(xr, ((0, 0), (1, 1), (1, 1)))
        pr = np.empty((9, C, L), np.float32)
        for di in range(3):
            for dj in range(3):
                pr[di * 3 + dj] = xrp[:, di:di + Hr, dj:dj + Wr].reshape(C, L)
        pr = pr.reshape(F, L)
        denom = np.sqrt((pr * pr).sum(0, dtype=np.float64).astype(np.float32)
                        + np.float32(F * EPS))

        mr = mask[b, :, ::RATE, ::RATE]
        mrp = np.pad(mr, ((0, 0), (1, 1), (1, 1)))
        msum = np.zeros((1, L), np.float32)
        for di in range(3):
            for dj in range(3):
                msum += mrp[:, di:di + Hr, dj:dj + Wr].reshape(1, L)
        mfilt = (msum[0] == 0.0).astype(np.float32)

        wn = (pr / denom[None, :]) * mfilt[None, :]

        # global softmax shift: exact after colsum normalization as long as
        # exp stays in f32 range; diag scores are ~denom_q so the midpoint
        # keeps args within +-5*spread
        cshift = 0.5 * float(denom.max() + denom.min())

        xp = np.pad(x[b], ((0, 0), (1, 1), (1, 1)))
        xu = np.empty((L, 16, C), np.float32)
        for i in range(4):
            for j in range(4):
                blk = xp[:, i:i + 2 * Hr:2, j:j + 2 * Wr:2]
                xu[:, i * 4 + j, :] = blk.reshape(C, L).T
        out.append((np.ascontiguousarray(wn.astype(NPBF16)),
                    pr, cshift, mfilt,
                    np.ascontiguousarray(
                        xu.reshape(L, CK).astype(NPBF16))))
    return out


def _col2im(col):
    """col [L, CK] -> [C, H, W] overlap-add, /4."""
    canvas = np.zeros((C, H + 2, W + 2), np.float32)
    blk = col.reshape(Hr, Wr, 16, C)
    for i in range(4):
        for j in range(4):
            canvas[:, i:i + 2 * Hr:2, j:j + 2 * Wr:2] += \
                blk[:, :, i * 4 + j, :].transpose(2, 0, 1)
    return canvas[:, 1:1 + H, 1:1 + W] / 4.0


def kernel(x, mask):
    x = np.asarray(x, np.float32)
    mask = np.asarray(mask, np.float32)
    if "nc" not in _CACHE:
        _CACHE["nc"] = _build_nc()
    nc = _CACHE["nc"]

    prep = _host_prep(x, mask)
    in_maps = []
    for core in range(N_CORES):
        b, g = divmod(core, 4)
        wn, pr, cshift, mfilt, xu = prep[b]
        q0 = g * QPC
        in_maps.append({
            "wn": wn,
            "prq": np.ascontiguousarray(pr[:, q0:q0 + QPC].astype(NPBF16)),
            "xu": xu,
            "mrow": np.ascontiguousarray(
                ((mfilt - 1.0) * 1e4 - SCALE * cshift)
                .astype(np.float32).reshape(32, 128).T),
        })

    _CACHE["in_maps"] = in_maps
    res = run_bass_kernel_spmd(nc, in_maps, list(range(N_CORES)))

    out = np.empty((B, C, H, W), np.float32)
    for b in range(B):
        col = np.concatenate(
            [res.results[b * 4 + g]["col"] for g in range(4)], axis=0)
        out[b] = _col2im(col)
    return out


# revision 6
# speedup vs baseline: 1.4779x; 1.3161x over previous
"""ContextualAttention TRN2 kernel.

Full inputs -> full output. Sharding: 8 cores = 2 batches x 4 q-blocks of the
L=4096 attention-column dimension. Each core computes, for its 1024 columns q:

  S[p, q]  = sum_f wn[f, p] * pr[f, q]          (QK^T, K = 1152 = 9 x 128)
  E[p, q]  = exp(10 * S - 10*c + m_p)           (c = global shift; any per-q
                                                 factor cancels in the colsum
                                                 normalization, so a single
                                                 constant that keeps exp in
                                                 f32 range is exact)
  A[p, q]  = E * mfilt_p                         (post-softmax patch mask)
  colT[q,:] = (A^T @ xu) / colsum_q              (conv_transpose as GEMM)

Host side: unfold / normalization prep (pure index shuffles + one divide) and
the final col2im overlap-add.  wn has the pre-softmax mask and 1/denom_p
folded in on the host.  All GEMM operands travel as bf16.

Schedule notes: every stationary (weight) tile feeds two 512-wide matmuls
back to back to amortize LDWEIGHTS; phase C walks q-tiles one at a time with
two PSUM banks each so bank recycling never stalls the PE; xu tiles are kept
resident across the 8 q-tile passes of each 1024-wide ck chunk and prefetched
on the scalar DMA queue.
"""
import numpy as np
import ml_dtypes

import concourse.bass as bass
import concourse.bacc as bacc
import concourse.mybir as mybir
from concourse import tile
from concourse.bass_utils import run_bass_kernel_spmd

F32 = mybir.dt.float32
BF16 = mybir.dt.bfloat16
AFT = mybir.ActivationFunctionType

B, C, H, W = 2, 128, 128, 128
RATE, BS = 2, 3                # attention rate, block size
Hr, Wr = H // RATE, W // RATE  # 64, 64
L = Hr * Wr                    # 4096
F = C * BS * BS                # 1152 contraction dim, 9 k-tiles
CK = C * 16                    # 2048 deconv output cols (kappa*128 + c)
QPC = L // 4                   # 1024 q columns per core
EPS = 1e-4
SCALE = 10.0
N_CORES = 8
NPBF16 = ml_dtypes.bfloat16

_CACHE = {}


def _build_nc():
    nc = bacc.Bacc(None)
    wn_d = nc.declare_dram_parameter("wn", [F, L], BF16, isOutput=False)
    prq_d = nc.declare_dram_parameter("prq", [F, QPC], BF16, isOutput=False)
    xu_d = nc.declare_dram_parameter("xu", [L, CK], BF16, isOutput=False)
    mrow_d = nc.declare_dram_parameter("mrow", [128, 32], F32, isOutput=False)
    col_d = nc.declare_dram_parameter("col", [QPC, CK], F32, isOutput=True)

    NPT = L // 128    # 32 p tiles
    NKT = F // 128    # 9 k tiles
    NQT = QPC // 128  # 8 q tiles
    NCH = CK // 1024  # 2 ck chunks of 1024

    with tile.TileContext(nc) as tc:
        with (
            tc.tile_pool(name="apool", bufs=NPT) as apool,
            tc.tile_pool(name="const", bufs=1) as cpool,
            tc.tile_pool(name="rhs", bufs=1) as rhspool,
            tc.tile_pool(name="lhs", bufs=3) as lhspool,
            tc.tile_pool(name="xus", bufs=40) as xupool,
            tc.tile_pool(name="outs", bufs=2) as opool,
            tc.tile_pool(name="rows", bufs=2) as rowpool,
            tc.tile_pool(name="ps", bufs=8, space="PSUM") as pspool,
        ):
            # ---- resident loads -------------------------------------------
            # per-k chunks so the first matmul only waits for its own slice
            rhs_sb = rhspool.tile([128, NKT * QPC], BF16)       # 18 KB/part
            rhs_r = rhs_sb[:].rearrange("p (k q) -> p k q", k=NKT)
            prq_r = prq_d[:].rearrange("(k fi) q -> fi k q", fi=128)
            for k in range(NKT):
                (nc.sync if k % 2 else nc.gpsimd).dma_start(
                    rhs_r[:, k], prq_r[:, k])
            m_sb = cpool.tile([128, 32], F32)
            nc.sync.dma_start(m_sb[:], mrow_d[:])
            ones_col = cpool.tile([128, 1], F32)
            nc.gpsimd.memset(ones_col[:], 1.0)
            acc = cpool.tile([128, QPC], F32)
            nc.gpsimd.memset(acc[:], 0.0)
            r8 = cpool.tile([128, NQT], F32)

            # prefetch first ck chunk of xu on the scalar queue during A
            xts = {}
            for pt in range(NPT):
                xt = xupool.tile([128, 1024], BF16, tag="xt",
                                 name=f"xt0_{pt}")
                nc.scalar.dma_start(xt[:], xu_d[pt * 128:(pt + 1) * 128,
                                                0:1024])
                xts[(0, pt)] = xt

            # ---- phase A: S = wn^T @ prq, E = exp(10 S + bias), acc += E --
            # each k weight tile feeds both 512-wide q chunks back to back
            a_tiles = []
            for pt in range(NPT):
                lhs = lhspool.tile([128, NKT * 128], BF16)
                (nc.gpsimd if pt % 2 else nc.sync).dma_start(
                    lhs[:].rearrange("p (k j) -> p k j", k=NKT),
                    wn_d[:, pt * 128:(pt + 1) * 128]
                    .rearrange("(k fi) j -> fi k j", fi=128))
                at = apool.tile([128, QPC], BF16)
                ps0 = pspool.tile([128, 512], F32, tag="ps")
                ps1 = pspool.tile([128, 512], F32, tag="ps")
                for k in range(NKT):
                    w = lhs[:, k * 128:(k + 1) * 128]
                    nc.tensor.matmul(ps0[:], w, rhs_sb[:, k * QPC:
                                                       k * QPC + 512],
                                     start=(k == 0), stop=(k == NKT - 1))
                    nc.tensor.matmul(ps1[:], w, rhs_sb[:, k * QPC + 512:
                                                       k * QPC + 1024],
                                     start=(k == 0), stop=(k == NKT - 1))
                nc.scalar.activation(at[:, 0:512], ps0[:], AFT.Exp,
                                     bias=m_sb[:, pt:pt + 1], scale=SCALE)
                nc.scalar.activation(at[:, 512:1024], ps1[:], AFT.Exp,
                                     bias=m_sb[:, pt:pt + 1], scale=SCALE)
                nc.vector.tensor_add(acc[:], acc[:], at[:])
                a_tiles.append(at)

            # ---- phase C: colT[q, ck] = sum_p A[p, q] xu[p, ck], scaled ---
            # (phase B colsum matmuls are slotted in after the first q-tile's
            #  matmuls so the PE never idles at the A->C boundary)
            first = True
            for ch in range(NCH):
                for qt in range(NQT):
                    psa = pspool.tile([128, 512], F32, tag="ps",
                                      name=f"psc{ch}_{qt}a")
                    psb = pspool.tile([128, 512], F32, tag="ps",
                                      name=f"psc{ch}_{qt}b")
                    for pt in range(NPT):
                        if (ch, pt) not in xts:
                            xt = xupool.tile([128, 1024], BF16, tag="xt",
                                             name=f"xt{ch}_{pt}")
                            (nc.gpsimd if pt % 2 else nc.sync).dma_start(
                                xt[:], xu_d[pt * 128:(pt + 1) * 128,
                                            ch * 1024:(ch + 1) * 1024])
                            xts[(ch, pt)] = xt
                        xt = xts[(ch, pt)]
                        w = a_tiles[pt][:, qt * 128:(qt + 1) * 128]
                        nc.tensor.matmul(psa[:], w, xt[:, 0:512],
                                         start=(pt == 0), stop=(pt == NPT - 1))
                        nc.tensor.matmul(psb[:], w, xt[:, 512:1024],
                                         start=(pt == 0), stop=(pt == NPT - 1))
                    if first:
                        # ---- phase B: r8[i, j] = 1/colsum(q=j*128+i) ------
                        first = False
                        cs_ps = pspool.tile([128, NQT], F32, tag="ps",
                                            name="csps")
                        for j in range(NQT):
                            nc.tensor.matmul(
                                cs_ps[:, j:j + 1],
                                acc[:, j * 128:(j + 1) * 128], ones_col[:],
                                start=True, stop=True)
                        nc.vector.tensor_copy(r8[:], cs_ps[:])
                        nc.vector.reciprocal(r8[:], r8[:])
                    ot = opool.tile([128, 1024], F32)
                    nc.vector.tensor_scalar_mul(ot[:, 0:512], psa[:],
                                                r8[:, qt:qt + 1])
                    nc.vector.tensor_scalar_mul(ot[:, 512:1024], psb[:],
                                                r8[:, qt:qt + 1])
                    nc.scalar.dma_start(
                        col_d[qt * 128:(qt + 1) * 128,
                              ch * 1024:(ch + 1) * 1024], ot[:])
                # chunk done: its xu tiles recycle via the pool
                for pt in range(NPT):
                    del xts[(ch, pt)]
    nc.compile()
    return nc


def _host_prep(x, mask):
    """Per-batch GEMM-ready operands (kappa-major feature layout)."""
    out = []
    for b in range(B):
        xr = x[b, :, ::RATE, ::RATE]
        xrp = np.pad(xr, ((0, 0), (1, 1), (1, 1)))
        pr = np.empty((9, C, L), np.float32)
        for di in range(3):
            for dj in range(3):
                pr[di * 3 + dj] = xrp[:, di:di + Hr, dj:dj + Wr].reshape(C, L)
        pr = pr.reshape(F, L)
        denom = np.sqrt((pr * pr).sum(0, dtype=np.float64).astype(np.float32)
                        + np.float32(F * EPS))

        mr = mask[b, :, ::RATE, ::RATE]
        mrp = np.pad(mr, ((0, 0), (1, 1), (1, 1)))
        msum = np.zeros((1, L), np.float32)
        for di in range(3):
            for dj in range(3):
                msum += mrp[:, di:di + Hr, dj:dj + Wr].reshape(1, L)
        mfilt = (msum[0] == 0.0).astype(np.float32)

        wn = (pr / denom[None, :]) * mfilt[None, :]

        # global softmax shift: exact after colsum normalization as long as
        # exp stays in f32 range; diag scores are ~denom_q so the midpoint
        # keeps args within +-5*spread
        cshift = 0.5 * float(denom.max() + denom.min())

        xp = np.pad(x[b], ((0, 0), (1, 1), (1, 1)))
        xu = np.empty((L, 16, C), np.float32)
        for i in range(4):
            for j in range(4):
                blk = xp[:, i:i + 2 * Hr:2, j:j + 2 * Wr:2]
                xu[:, i * 4 + j, :] = blk.reshape(C, L).T
        out.append((np.ascontiguousarray(wn.astype(NPBF16)),
                    pr, cshift, mfilt,
                    np.ascontiguousarray(
                        xu.reshape(L, CK).astype(NPBF16))))
    return out


def _col2im(col):
    """col [L, CK] -> [C, H, W] overlap-add, /4."""
    canvas = np.zeros((C, H + 2, W + 2), np.float32)
    blk = col.reshape(Hr, Wr, 16, C)
    for i in range(4):
        for j in range(4):
            canvas[:, i:i + 2 * Hr:2, j:j + 2 * Wr:2] += \
                blk[:, :, i * 4 + j, :].transpose(2, 0, 1)
    return canvas[:, 1:1 + H, 1:1 + W] / 4.0


def kernel(x, mask):
    x = np.asarray(x, np.float32)
    mask = np.asarray(mask, np.float32)
    if "nc" not in _CACHE:
        _CACHE["nc"] = _build_nc()
    nc = _CACHE["nc"]

    prep = _host_prep(x, mask)
    in_maps = []
    for core in range(N_CORES):
        b, g = divmod(core, 4)
        wn, pr, cshift, mfilt, xu = prep[b]
        q0 = g * QPC
        in_maps.append({
            "wn": wn,
            "prq": np.ascontiguousarray(pr[:, q0:q0 + QPC].astype(NPBF16)),
            "xu": xu,
            "mrow": np.ascontiguousarray(
                ((mfilt - 1.0) * 1e4 - SCALE * cshift)
                .astype(np.float32).reshape(32, 128).T),
        })

    _CACHE["in_maps"] = in_maps
    res = run_bass_kernel_spmd(nc, in_maps, list(range(N_CORES)))

    out = np.empty((B, C, H, W), np.float32)
    for b in range(B):
        col = np.concatenate(
            [res.results[b * 4 + g]["col"] for g in range(4)], axis=0)
        out[b] = _col2im(col)
    return out


# revision 12
# speedup vs baseline: 1.5163x; 1.0260x over previous
"""ContextualAttention TRN2 kernel.

Full inputs -> full output. Sharding: 8 cores = 2 batches x 4 q-blocks of the
L=4096 attention-column dimension. Each core computes, for its 1024 columns q:

  S[p, q]  = sum_f wn[f, p] * pr[f, q]          (QK^T, K = 1152 = 9 x 128)
  E[p, q]  = exp(10 * S - 10*c + m_p)           (c = global shift; any per-q
                                                 factor cancels in the colsum
                                                 normalization, so a single
                                                 constant that keeps exp in
                                                 f32 range is exact)
  A[p, q]  = E * mfilt_p                         (post-softmax patch mask)
  colT[q,:] = (A^T @ xu) / colsum_q              (conv_transpose as GEMM)

Host side: unfold / normalization prep (pure index shuffles + one divide) and
the final col2im overlap-add.  wn has the pre-softmax mask and 1/denom_p
folded in on the host.  All GEMM operands travel as bf16.

Schedule notes: every stationary (weight) tile feeds two 512-wide matmuls
back to back to amortize LDWEIGHTS; phase C walks q-tiles one at a time with
two PSUM banks each so bank recycling never stalls the PE; xu tiles are kept
resident across the 8 q-tile passes of each 1024-wide ck chunk and prefetched
on the scalar DMA queue.
"""
import numpy as np
import ml_dtypes

import concourse.bass as bass
import concourse.bacc as bacc
import concourse.mybir as mybir
from concourse import tile
from concourse.bass_utils import run_bass_kernel_spmd

F32 = mybir.dt.float32
BF16 = mybir.dt.bfloat16
AFT = mybir.ActivationFunctionType

B, C, H, W = 2, 128, 128, 128
RATE, BS = 2, 3                # attention rate, block size
Hr, Wr = H // RATE, W // RATE  # 64, 64
L = Hr * Wr                    # 4096
F = C * BS * BS                # 1152 contraction dim, 9 k-tiles
CK = C * 16                    # 2048 deconv output cols (kappa*128 + c)
QPC = L // 4                   # 1024 q columns per core
EPS = 1e-4
SCALE = 10.0
N_CORES = 8
NPBF16 = ml_dtypes.bfloat16

_CACHE = {}


def _build_nc():
    nc = bacc.Bacc(None)
    wn_d = nc.declare_dram_parameter("wn", [F, L], BF16, isOutput=False)
    prq_d = nc.declare_dram_parameter("prq", [F, QPC], BF16, isOutput=False)
    xu_d = nc.declare_dram_parameter("xu", [L, CK], BF16, isOutput=False)
    mrow_d = nc.declare_dram_parameter("mrow", [128, 32], F32, isOutput=False)
    col_d = nc.declare_dram_parameter("col", [QPC, CK], BF16, isOutput=True)

    NPT = L // 128    # 32 p tiles
    NKT = F // 128    # 9 k tiles
    NQT = QPC // 128  # 8 q tiles
    NCH = CK // 1024  # 2 ck chunks of 1024

    with tile.TileContext(nc) as tc:
        with (
            tc.tile_pool(name="apool", bufs=NPT) as apool,
            tc.tile_pool(name="const", bufs=1) as cpool,
            tc.tile_pool(name="rhs", bufs=1) as rhspool,
            tc.tile_pool(name="lhs", bufs=3) as lhspool,
            tc.tile_pool(name="xus", bufs=40) as xupool,
            tc.tile_pool(name="outs", bufs=2) as opool,
            tc.tile_pool(name="rows", bufs=2) as rowpool,
            tc.tile_pool(name="ps", bufs=8, space="PSUM") as pspool,
        ):
            # ---- resident loads -------------------------------------------
            # per-k chunks so the first matmul only waits for its own slice;
            # the k=0 chunk and the first lhs tile go first on the sync queue
            rhs_sb = rhspool.tile([128, NKT * QPC], BF16)       # 18 KB/part
            rhs_r = rhs_sb[:].rearrange("p (k q) -> p k q", k=NKT)
            prq_r = prq_d[:].rearrange("(k fi) q -> fi k q", fi=128)
            nc.sync.dma_start(rhs_r[:, 0], prq_r[:, 0])
            lhs0 = lhspool.tile([128, NKT * 128], BF16, tag="lhs")
            nc.sync.dma_start(
                lhs0[:].rearrange("p (k j) -> p k j", k=NKT),
                wn_d[:, 0:128].rearrange("(k fi) j -> fi k j", fi=128))
            for k in range(1, NKT):
                (nc.sync if k % 2 else nc.gpsimd).dma_start(
                    rhs_r[:, k], prq_r[:, k])
            m_sb = cpool.tile([128, 32], F32)
            nc.gpsimd.dma_start(m_sb[:], mrow_d[:])
            ones_col = cpool.tile([128, 1], F32)
            nc.gpsimd.memset(ones_col[:], 1.0)
            acc = cpool.tile([128, QPC], F32)
            nc.gpsimd.memset(acc[:], 0.0)
            r8 = cpool.tile([128, NQT], F32)

            # ---- phase A: S = wn^T @ prq, E = exp(10 S + bias), acc += E --
            # each k weight tile feeds both 512-wide q chunks back to back;
            # xu's first ck chunk prefetches on the vector queue meanwhile
            xts = {}
            a_tiles = []
            for pt in range(NPT):
                if pt == 0:
                    lhs = lhs0
                else:
                    lhs = lhspool.tile([128, NKT * 128], BF16, tag="lhs")
                    (nc.gpsimd if pt % 2 else nc.sync).dma_start(
                        lhs[:].rearrange("p (k j) -> p k j", k=NKT),
                        wn_d[:, pt * 128:(pt + 1) * 128]
                        .rearrange("(k fi) j -> fi k j", fi=128))
                at = apool.tile([128, QPC], BF16)
                ps0 = pspool.tile([128, 512], F32, tag="ps")
                ps1 = pspool.tile([128, 512], F32, tag="ps")
                for k in range(NKT):
                    w = lhs[:, k * 128:(k + 1) * 128]
                    nc.tensor.matmul(ps0[:], w, rhs_sb[:, k * QPC:
                                                       k * QPC + 512],
                                     start=(k == 0), stop=(k == NKT - 1))
                    nc.tensor.matmul(ps1[:], w, rhs_sb[:, k * QPC + 512:
                                                       k * QPC + 1024],
                                     start=(k == 0), stop=(k == NKT - 1))
                nc.scalar.activation(at[:, 0:512], ps0[:], AFT.Exp,
                                     bias=m_sb[:, pt:pt + 1], scale=SCALE)
                nc.scalar.activation(at[:, 512:1024], ps1[:], AFT.Exp,
                                     bias=m_sb[:, pt:pt + 1], scale=SCALE)
                nc.vector.tensor_add(acc[:], acc[:], at[:])
                xt = xupool.tile([128, 1024], BF16, tag="xt",
                                 name=f"xt0_{pt}")
                (nc.sync if pt % 2 else nc.gpsimd).dma_start(
                    xt[:], xu_d[pt * 128:(pt + 1) * 128, 0:1024])
                xts[(0, pt)] = xt
                a_tiles.append(at)

            # ---- phase C: colT[q, ck] = sum_p A[p, q] xu[p, ck], scaled ---
            # (phase B colsum matmuls are slotted in after the first q-tile's
            #  matmuls so the PE never idles at the A->C boundary)
            first = True
            for ch in range(NCH):
                for qt in range(NQT):
                    psa = pspool.tile([128, 512], F32, tag="ps",
                                      name=f"psc{ch}_{qt}a")
                    psb = pspool.tile([128, 512], F32, tag="ps",
                                      name=f"psc{ch}_{qt}b")
                    for pt in range(NPT):
                        if (ch, pt) not in xts:
                            xt = xupool.tile([128, 1024], BF16, tag="xt",
                                             name=f"xt{ch}_{pt}")
                            (nc.gpsimd if pt % 2 else nc.sync).dma_start(
                                xt[:], xu_d[pt * 128:(pt + 1) * 128,
                                            ch * 1024:(ch + 1) * 1024])
                            xts[(ch, pt)] = xt
                        xt = xts[(ch, pt)]
                        w = a_tiles[pt][:, qt * 128:(qt + 1) * 128]
                        nc.tensor.matmul(psa[:], w, xt[:, 0:512],
                                         start=(pt == 0), stop=(pt == NPT - 1))
                        nc.tensor.matmul(psb[:], w, xt[:, 512:1024],
                                         start=(pt == 0), stop=(pt == NPT - 1))
                    if first:
                        # ---- phase B: r8[i, j] = 1/colsum(q=j*128+i) ------
                        first = False
                        cs_ps = pspool.tile([128, NQT], F32, tag="ps",
                                            name="csps")
                        for j in range(NQT):
                            nc.tensor.matmul(
                                cs_ps[:, j:j + 1],
                                acc[:, j * 128:(j + 1) * 128], ones_col[:],
                                start=True, stop=True)
                        nc.vector.tensor_copy(r8[:], cs_ps[:])
                        nc.vector.reciprocal(r8[:], r8[:])
                    ot = opool.tile([128, 1024], BF16)
                    nc.vector.tensor_scalar_mul(ot[:, 0:512], psa[:],
                                                r8[:, qt:qt + 1])
                    nc.vector.tensor_scalar_mul(ot[:, 512:1024], psb[:],
                                                r8[:, qt:qt + 1])
                    nc.scalar.dma_start(
                        col_d[qt * 128:(qt + 1) * 128,
                              ch * 1024:ch * 1024 + 512], ot[:, 0:512])
                    nc.gpsimd.dma_start(
                        col_d[qt * 128:(qt + 1) * 128,
                              ch * 1024 + 512:(ch + 1) * 1024],
                        ot[:, 512:1024])
                # chunk done: its xu tiles recycle via the pool
                for pt in range(NPT):
                    del xts[(ch, pt)]
    nc.compile()
    return nc


def _host_prep(x, mask):
    """Per-batch GEMM-ready operands (kappa-major feature layout)."""
    out = []
    for b in range(B):
        xr = x[b, :, ::RATE, ::RATE]
        xrp = np.pad(xr, ((0, 0), (1, 1), (1, 1)))
        pr = np.empty((9, C, L), np.float32)
        for di in range(3):
            for dj in range(3):
                pr[di * 3 + dj] = xrp[:, di:di + Hr, dj:dj + Wr].reshape(C, L)
        pr = pr.reshape(F, L)
        denom = np.sqrt((pr * pr).sum(0, dtype=np.float64).astype(np.float32)
                        + np.float32(F * EPS))

        mr = mask[b, :, ::RATE, ::RATE]
        mrp = np.pad(mr, ((0, 0), (1, 1), (1, 1)))
        msum = np.zeros((1, L), np.float32)
        for di in range(3):
            for dj in range(3):
                msum += mrp[:, di:di + Hr, dj:dj + Wr].reshape(1, L)
        mfilt = (msum[0] == 0.0).astype(np.float32)

        wn = (pr / denom[None, :]) * mfilt[None, :]

        # global softmax shift: exact after colsum normalization as long as
        # exp stays in f32 range; diag scores are ~denom_q so the midpoint
        # keeps args within +-5*spread
        cshift = 0.5 * float(denom.max() + denom.min())

        xp = np.pad(x[b], ((0, 0), (1, 1), (1, 1)))
        xu = np.empty((L, 16, C), np.float32)
        for i in range(4):
            for j in range(4):
                blk = xp[:, i:i + 2 * Hr:2, j:j + 2 * Wr:2]
                xu[:, i * 4 + j, :] = blk.reshape(C, L).T
        out.append((np.ascontiguousarray(wn.astype(NPBF16)),
                    pr, cshift, mfilt,
                    np.ascontiguousarray(
                        xu.reshape(L, CK).astype(NPBF16))))
    return out


def _col2im(col):
    """col [L, CK] -> [C, H, W] overlap-add, /4."""
    canvas = np.zeros((C, H + 2, W + 2), np.float32)
    blk = col.reshape(Hr, Wr, 16, C)
    for i in range(4):
        for j in range(4):
            canvas[:, i:i + 2 * Hr:2, j:j + 2 * Wr:2] += \
                blk[:, :, i * 4 + j, :].transpose(2, 0, 1)
    return canvas[:, 1:1 + H, 1:1 + W] / 4.0


def kernel(x, mask):
    x = np.asarray(x, np.float32)
    mask = np.asarray(mask, np.float32)
    if "nc" not in _CACHE:
        _CACHE["nc"] = _build_nc()
    nc = _CACHE["nc"]

    prep = _host_prep(x, mask)
    in_maps = []
    for core in range(N_CORES):
        b, g = divmod(core, 4)
        wn, pr, cshift, mfilt, xu = prep[b]
        q0 = g * QPC
        in_maps.append({
            "wn": wn,
            "prq": np.ascontiguousarray(pr[:, q0:q0 + QPC].astype(NPBF16)),
            "xu": xu,
            "mrow": np.ascontiguousarray(
                ((mfilt - 1.0) * 1e4 - SCALE * cshift)
                .astype(np.float32).reshape(32, 128).T),
        })

    _CACHE["in_maps"] = in_maps
    res = run_bass_kernel_spmd(nc, in_maps, list(range(N_CORES)))

    out = np.empty((B, C, H, W), np.float32)
    for b in range(B):
        col = np.concatenate(
            [np.asarray(res.results[b * 4 + g]["col"], np.float32)
             for g in range(4)], axis=0)
        out[b] = _col2im(col)
    return out


# revision 20
# speedup vs baseline: 1.7519x; 1.1554x over previous
"""ContextualAttention TRN2 kernel.

Full inputs -> full output. Sharding: 8 cores = 2 batches x 4 q-blocks of the
L=4096 attention-column dimension. Each core computes, for its 1024 columns q:

  S[p, q]  = sum_f wn[f, p] * pr[f, q]          (QK^T, K = 1152 = 9 x 128)
  E[p, q]  = exp(10 * S - 10*c + m_p)           (c = global shift; any per-q
                                                 factor cancels in the colsum
                                                 normalization, so a single
                                                 constant that keeps exp in
                                                 f32 range is exact)
  A[p, q]  = E * mfilt_p                         (post-softmax patch mask)
  colT[q,:] = (A^T @ xu) / colsum_q              (conv_transpose as GEMM)

Host side: unfold / normalization prep (pure index shuffles + one divide) and
the final col2im overlap-add.  wn has the pre-softmax mask and 1/denom_p
folded in on the host.  All GEMM operands travel as bf16.

Schedule notes: every stationary (weight) tile feeds two 512-wide matmuls
back to back to amortize LDWEIGHTS; phase C walks q-tiles one at a time with
two PSUM banks each so bank recycling never stalls the PE; xu tiles are kept
resident across the 8 q-tile passes of each 1024-wide ck chunk and prefetched
on the scalar DMA queue.
"""
import numpy as np
import ml_dtypes

import concourse.bass as bass
import concourse.bacc as bacc
import concourse.mybir as mybir
from concourse import tile
from concourse.bass_utils import run_bass_kernel_spmd

F32 = mybir.dt.float32
BF16 = mybir.dt.bfloat16
FP8 = mybir.dt.float8e4
DR = mybir.MatmulPerfMode.DoubleRow
AFT = mybir.ActivationFunctionType

B, C, H, W = 2, 128, 128, 128
RATE, BS = 2, 3                # attention rate, block size
Hr, Wr = H // RATE, W // RATE  # 64, 64
L = Hr * Wr                    # 4096
F = C * BS * BS                # 1152 contraction dim, 9 k-tiles
CK = C * 16                    # 2048 deconv output cols (kappa*128 + c)
QPC = L // 4                   # 1024 q columns per core
EPS = 1e-4
SCALE = 10.0
N_CORES = 8
NPBF16 = ml_dtypes.bfloat16
NPFP8 = ml_dtypes.float8_e4m3
WNS = 16.0   # fp8 pre-scale on wn, undone by the activation scale

_CACHE = {}


def _build_nc():
    nc = bacc.Bacc(None)
    wn_d = nc.declare_dram_parameter("wn", [F, L], FP8, isOutput=False)
    prq_d = nc.declare_dram_parameter("prq", [F, QPC], FP8, isOutput=False)
    xu_d = nc.declare_dram_parameter("xu", [L, CK], BF16, isOutput=False)
    mrow_d = nc.declare_dram_parameter("mrow", [128, 32], F32, isOutput=False)
    col_d = nc.declare_dram_parameter("col", [QPC, CK], BF16, isOutput=True)

    NPT = L // 128    # 32 p tiles
    NKT = F // 128    # 9 k tiles
    NQT = QPC // 128  # 8 q tiles
    NCH = CK // 1024  # 2 ck chunks of 1024

    with tile.TileContext(nc) as tc:
        with (
            tc.tile_pool(name="apool", bufs=NPT) as apool,
            tc.tile_pool(name="const", bufs=1) as cpool,
            tc.tile_pool(name="rhs", bufs=1) as rhspool,
            tc.tile_pool(name="lhs", bufs=3) as lhspool,
            tc.tile_pool(name="xus", bufs=40) as xupool,
            tc.tile_pool(name="outs", bufs=2) as opool,
            tc.tile_pool(name="rows", bufs=2) as rowpool,
            tc.tile_pool(name="ps", bufs=8, space="PSUM") as pspool,
        ):
            # ---- resident loads -------------------------------------------
            # per-k chunks so the first matmul only waits for its own slice;
            # the k=0 chunk and the first lhs tile go first on the sync queue
            rhs_sb = rhspool.tile([128, NKT * QPC], FP8)        # 9 KB/part
            rhs_r = rhs_sb[:].rearrange("p (k q) -> p k q", k=NKT)
            prq_r = prq_d[:].rearrange("(k fi) q -> fi k q", fi=128)
            nc.sync.dma_start(rhs_r[:, 0], prq_r[:, 0])
            lhs0 = lhspool.tile([128, NKT * 128], FP8, tag="lhs")
            nc.sync.dma_start(
                lhs0[:].rearrange("p (k j) -> p k j", k=NKT),
                wn_d[:, 0:128].rearrange("(k fi) j -> fi k j", fi=128))
            for k in range(1, NKT):
                (nc.sync if k % 2 else nc.gpsimd).dma_start(
                    rhs_r[:, k], prq_r[:, k])
            m_sb = cpool.tile([128, 32], F32)
            nc.gpsimd.dma_start(m_sb[:], mrow_d[:])
            ones_col = cpool.tile([128, 1], F32)
            nc.gpsimd.memset(ones_col[:], 1.0)
            acc = cpool.tile([128, QPC], F32)
            nc.gpsimd.memset(acc[:], 0.0)
            r8 = cpool.tile([128, NQT], F32)

            # ---- phase A: S = wn^T @ prq, E = exp(10 S + bias), acc += E --
            # each k weight tile feeds both 512-wide q chunks back to back;
            # xu's first ck chunk prefetches on the vector queue meanwhile
            xts = {}
            a_tiles = []
            for pt in range(NPT):
                if pt == 0:
                    lhs = lhs0
                else:
                    lhs = lhspool.tile([128, NKT * 128], FP8, tag="lhs")
                    (nc.gpsimd if pt % 2 else nc.sync).dma_start(
                        lhs[:].rearrange("p (k j) -> p k j", k=NKT),
                        wn_d[:, pt * 128:(pt + 1) * 128]
                        .rearrange("(k fi) j -> fi k j", fi=128))
                at = apool.tile([128, QPC], BF16)
                ps0 = pspool.tile([128, 512], F32, tag="ps")
                ps1 = pspool.tile([128, 512], F32, tag="ps")
                lhs_r = lhs[:].rearrange("p (k j) -> p k j", k=NKT)
                # 4 DoubleRow matmuls contract k-pairs, a plain fp8 matmul
                # takes the 9th k-tile; each weight feeds both q chunks
                for kt in range(4):
                    w2 = lhs_r[:, 2 * kt:2 * kt + 2]
                    nc.tensor.matmul(ps0[:], w2, rhs_r[:, 2 * kt:2 * kt + 2,
                                                       0:512],
                                     start=(kt == 0), stop=False,
                                     perf_mode=DR)
                    nc.tensor.matmul(ps1[:], w2, rhs_r[:, 2 * kt:2 * kt + 2,
                                                       512:1024],
                                     start=(kt == 0), stop=False,
                                     perf_mode=DR)
                w = lhs[:, 8 * 128:9 * 128]
                nc.tensor.matmul(ps0[:], w, rhs_sb[:, 8 * QPC:8 * QPC + 512],
                                 start=False, stop=True)
                nc.tensor.matmul(ps1[:], w, rhs_sb[:, 8 * QPC + 512:
                                                   8 * QPC + 1024],
                                 start=False, stop=True)
                nc.scalar.activation(at[:, 0:512], ps0[:], AFT.Exp,
                                     bias=m_sb[:, pt:pt + 1],
                                     scale=SCALE / 16.0)
                nc.scalar.activation(at[:, 512:1024], ps1[:], AFT.Exp,
                                     bias=m_sb[:, pt:pt + 1],
                                     scale=SCALE / 16.0)
                nc.vector.tensor_add(acc[:], acc[:], at[:])
                # stagger xu prefetch 8 tiles behind to keep the early
                # queues clear for rhs/lhs
                if pt >= 8:
                    j = pt - 8
                    xt = xupool.tile([128, 1024], BF16, tag="xt",
                                     name=f"xt0_{j}")
                    (nc.sync if j % 2 else nc.gpsimd).dma_start(
                        xt[:], xu_d[j * 128:(j + 1) * 128, 0:1024])
                    xts[(0, j)] = xt
                a_tiles.append(at)
            for j in range(NPT - 8, NPT):
                xt = xupool.tile([128, 1024], BF16, tag="xt",
                                 name=f"xt0_{j}")
                (nc.sync if j % 2 else nc.gpsimd).dma_start(
                    xt[:], xu_d[j * 128:(j + 1) * 128, 0:1024])
                xts[(0, j)] = xt

            # ---- phase C: colT[q, ck] = sum_p A[p, q] xu[p, ck], scaled ---
            # (phase B colsum matmuls are slotted in after the first q-tile's
            #  matmuls so the PE never idles at the A->C boundary)
            first = True
            for ch in range(NCH):
                for qt in range(NQT):
                    psa = pspool.tile([128, 512], F32, tag="ps",
                                      name=f"psc{ch}_{qt}a")
                    psb = pspool.tile([128, 512], F32, tag="ps",
                                      name=f"psc{ch}_{qt}b")
                    for pt in range(NPT):
                        if (ch, pt) not in xts:
                            xt = xupool.tile([128, 1024], BF16, tag="xt",
                                             name=f"xt{ch}_{pt}")
                            (nc.gpsimd if pt % 2 else nc.sync).dma_start(
                                xt[:], xu_d[pt * 128:(pt + 1) * 128,
                                            ch * 1024:(ch + 1) * 1024])
                            xts[(ch, pt)] = xt
                        xt = xts[(ch, pt)]
                        w = a_tiles[pt][:, qt * 128:(qt + 1) * 128]
                        nc.tensor.matmul(psa[:], w, xt[:, 0:512],
                                         start=(pt == 0), stop=(pt == NPT - 1))
                        nc.tensor.matmul(psb[:], w, xt[:, 512:1024],
                                         start=(pt == 0), stop=(pt == NPT - 1))
                    if first:
                        # ---- phase B: r8[i, j] = 1/colsum(q=j*128+i) ------
                        first = False
                        cs_ps = pspool.tile([128, NQT], F32, tag="ps",
                                            name="csps")
                        for j in range(NQT):
                            nc.tensor.matmul(
                                cs_ps[:, j:j + 1],
                                acc[:, j * 128:(j + 1) * 128], ones_col[:],
                                start=True, stop=True)
                        nc.vector.tensor_copy(r8[:], cs_ps[:])
                        nc.vector.reciprocal(r8[:], r8[:])
                    ot = opool.tile([128, 1024], BF16)
                    nc.vector.tensor_scalar_mul(ot[:, 0:512], psa[:],
                                                r8[:, qt:qt + 1])
                    nc.vector.tensor_scalar_mul(ot[:, 512:1024], psb[:],
                                                r8[:, qt:qt + 1])
                    if ch == NCH - 1 and qt == NQT - 1:
                        # final store fans out over three queues to cut the
                        # end-of-kernel drain tail
                        qs = (nc.scalar, nc.sync, nc.gpsimd, nc.scalar)
                        for si in range(4):
                            qs[si].dma_start(
                                col_d[qt * 128:(qt + 1) * 128,
                                      ch * 1024 + si * 256:
                                      ch * 1024 + (si + 1) * 256],
                                ot[:, si * 256:(si + 1) * 256])
                    else:
                        nc.scalar.dma_start(
                            col_d[qt * 128:(qt + 1) * 128,
                                  ch * 1024:ch * 1024 + 512], ot[:, 0:512])
                        nc.gpsimd.dma_start(
                            col_d[qt * 128:(qt + 1) * 128,
                                  ch * 1024 + 512:(ch + 1) * 1024],
                            ot[:, 512:1024])
                # chunk done: its xu tiles recycle via the pool
                for pt in range(NPT):
                    del xts[(ch, pt)]
    nc.compile()
    return nc


def _host_prep(x, mask):
    """Per-batch GEMM-ready operands (kappa-major feature layout)."""
    out = []
    for b in range(B):
        xr = x[b, :, ::RATE, ::RATE]
        xrp = np.pad(xr, ((0, 0), (1, 1), (1, 1)))
        pr = np.empty((9, C, L), np.float32)
        for di in range(3):
            for dj in range(3):
                pr[di * 3 + dj] = xrp[:, di:di + Hr, dj:dj + Wr].reshape(C, L)
        pr = pr.reshape(F, L)
        denom = np.sqrt((pr * pr).sum(0, dtype=np.float64).astype(np.float32)
                        + np.float32(F * EPS))

        mr = mask[b, :, ::RATE, ::RATE]
        mrp = np.pad(mr, ((0, 0), (1, 1), (1, 1)))
        msum = np.zeros((1, L), np.float32)
        for di in range(3):
            for dj in range(3):
                msum += mrp[:, di:di + Hr, dj:dj + Wr].reshape(1, L)
        mfilt = (msum[0] == 0.0).astype(np.float32)

        wn = (pr / denom[None, :]) * mfilt[None, :]

        # global softmax shift: exact after colsum normalization as long as
        # exp stays in f32 range; diag scores are ~denom_q so the midpoint
        # keeps args within +-5*spread
        cshift = 0.5 * float(denom.max() + denom.min())

        xp = np.pad(x[b], ((0, 0), (1, 1), (1, 1)))
        xu = np.empty((L, 16, C), np.float32)
        for i in range(4):
            for j in range(4):
                blk = xp[:, i:i + 2 * Hr:2, j:j + 2 * Wr:2]
                xu[:, i * 4 + j, :] = blk.reshape(C, L).T
        out.append((np.ascontiguousarray((wn * WNS).astype(NPFP8)),
                    pr, cshift, mfilt,
                    np.ascontiguousarray(
                        xu.reshape(L, CK).astype(NPBF16))))
    return out


def _col2im(col):
    """col [L, CK] -> [C, H, W] overlap-add, /4."""
    canvas = np.zeros((C, H + 2, W + 2), np.float32)
    blk = col.reshape(Hr, Wr, 16, C)
    for i in range(4):
        for j in range(4):
            canvas[:, i:i + 2 * Hr:2, j:j + 2 * Wr:2] += \
                blk[:, :, i * 4 + j, :].transpose(2, 0, 1)
    return canvas[:, 1:1 + H, 1:1 + W] / 4.0


def kernel(x, mask):
    x = np.asarray(x, np.float32)
    mask = np.asarray(mask, np.float32)
    if "nc" not in _CACHE:
        _CACHE["nc"] = _build_nc()
    nc = _CACHE["nc"]

    prep = _host_prep(x, mask)
    in_maps = []
    for core in range(N_CORES):
        b, g = divmod(core, 4)
        wn, pr, cshift, mfilt, xu = prep[b]
        q0 = g * QPC
        in_maps.append({
            "wn": wn,
            "prq": np.ascontiguousarray(pr[:, q0:q0 + QPC].astype(NPFP8)),
            "xu": xu,
            "mrow": np.ascontiguousarray(
                ((mfilt - 1.0) * 1e4 - SCALE * cshift)
                .astype(np.float32).reshape(32, 128).T),
        })

    _CACHE["in_maps"] = in_maps
    res = run_bass_kernel_spmd(nc, in_maps, list(range(N_CORES)))

    out = np.empty((B, C, H, W), np.float32)
    for b in range(B):
        col = np.concatenate(
            [np.asarray(res.results[b * 4 + g]["col"], np.float32)
             for g in range(4)], axis=0)
        out[b] = _col2im(col)
    return out


# revision 23
# speedup vs baseline: 1.7664x; 1.0082x over previous
"""ContextualAttention TRN2 kernel.

Full inputs -> full output. Sharding: 8 cores = 2 batches x 4 q-blocks of the
L=4096 attention-column dimension. Each core computes, for its 1024 columns q:

  S[p, q]  = sum_f wn[f, p] * pr[f, q]          (QK^T, K = 1152 = 9 x 128)
  E[p, q]  = exp(10 * S - 10*c + m_p)           (c = global shift; any per-q
                                                 factor cancels in the colsum
                                                 normalization, so a single
                                                 constant that keeps exp in
                                                 f32 range is exact)
  A[p, q]  = E * mfilt_p                         (post-softmax patch mask)
  colT[q,:] = (A^T @ xu) / colsum_q              (conv_transpose as GEMM)

Host side: unfold / normalization prep (pure index shuffles + one divide) and
the final col2im overlap-add.  wn has the pre-softmax mask and 1/denom_p
folded in on the host.  All GEMM operands travel as bf16.

Schedule notes: every stationary (weight) tile feeds two 512-wide matmuls
back to back to amortize LDWEIGHTS; phase C walks q-tiles one at a time with
two PSUM banks each so bank recycling never stalls the PE; xu tiles are kept
resident across the 8 q-tile passes of each 1024-wide ck chunk and prefetched
on the scalar DMA queue.
"""
import numpy as np
import ml_dtypes

import concourse.bass as bass
import concourse.bacc as bacc
import concourse.mybir as mybir
from concourse import tile
from concourse.bass_utils import run_bass_kernel_spmd

F32 = mybir.dt.float32
BF16 = mybir.dt.bfloat16
FP8 = mybir.dt.float8e4
DR = mybir.MatmulPerfMode.DoubleRow
AFT = mybir.ActivationFunctionType

B, C, H, W = 2, 128, 128, 128
RATE, BS = 2, 3                # attention rate, block size
Hr, Wr = H // RATE, W // RATE  # 64, 64
L = Hr * Wr                    # 4096
F = C * BS * BS                # 1152 contraction dim, 9 k-tiles
CK = C * 16                    # 2048 deconv output cols (kappa*128 + c)
QPC = L // 4                   # 1024 q columns per core
EPS = 1e-4
SCALE = 10.0
N_CORES = 8
NPBF16 = ml_dtypes.bfloat16
NPFP8 = ml_dtypes.float8_e4m3
WNS = 16.0   # fp8 pre-scale on wn, undone by the activation scale

_CACHE = {}


def _build_nc():
    nc = bacc.Bacc(None)
    wn_d = nc.declare_dram_parameter("wn", [F, L], FP8, isOutput=False)
    prq_d = nc.declare_dram_parameter("prq", [F, QPC], FP8, isOutput=False)
    xu_d = nc.declare_dram_parameter("xu", [L, CK], BF16, isOutput=False)
    mrow_d = nc.declare_dram_parameter("mrow", [128, 32], F32, isOutput=False)
    col_d = nc.declare_dram_parameter("col", [QPC, CK], BF16, isOutput=True)

    NPT = L // 128    # 32 p tiles
    NKT = F // 128    # 9 k tiles
    NQT = QPC // 128  # 8 q tiles
    NCH = CK // 1024  # 2 ck chunks of 1024

    with tile.TileContext(nc) as tc:
        with (
            tc.tile_pool(name="apool", bufs=NPT) as apool,
            tc.tile_pool(name="const", bufs=1) as cpool,
            tc.tile_pool(name="rhs", bufs=1) as rhspool,
            tc.tile_pool(name="lhs", bufs=3) as lhspool,
            tc.tile_pool(name="xus", bufs=40) as xupool,
            tc.tile_pool(name="outs", bufs=2) as opool,
            tc.tile_pool(name="rows", bufs=2) as rowpool,
            tc.tile_pool(name="ps", bufs=8, space="PSUM") as pspool,
        ):
            # ---- resident loads -------------------------------------------
            # per-k chunks so the first matmul only waits for its own slice;
            # the k=0 chunk and the first lhs tile go first on the sync queue
            rhs_sb = rhspool.tile([128, NKT * QPC], FP8)        # 9 KB/part
            rhs_r = rhs_sb[:].rearrange("p (k q) -> p k q", k=NKT)
            prq_r = prq_d[:].rearrange("(k fi) q -> fi k q", fi=128)
            lhs0 = lhspool.tile([128, NKT * 128], FP8, tag="lhs")
            nc.gpsimd.dma_start(
                lhs0[:].rearrange("p (k j) -> p k j", k=NKT),
                wn_d[:, 0:128].rearrange("(k fi) j -> fi k j", fi=128))
            for k in range(NKT):
                (nc.sync if k % 2 == 0 else nc.scalar).dma_start(
                    rhs_r[:, k], prq_r[:, k])
            m_sb = cpool.tile([128, 32], F32)
            nc.gpsimd.dma_start(m_sb[:], mrow_d[:])
            ones_col = cpool.tile([128, 1], F32)
            nc.gpsimd.memset(ones_col[:], 1.0)
            acc = cpool.tile([128, QPC], F32)
            nc.gpsimd.memset(acc[:], 0.0)
            r8 = cpool.tile([128, NQT], F32)

            # ---- phase A: S = wn^T @ prq, E = exp(10 S + bias), acc += E --
            # each k weight tile feeds both 512-wide q chunks back to back;
            # xu's first ck chunk prefetches on the vector queue meanwhile
            xts = {}
            a_tiles = []
            for pt in range(NPT):
                if pt == 0:
                    lhs = lhs0
                else:
                    lhs = lhspool.tile([128, NKT * 128], FP8, tag="lhs")
                    (nc.gpsimd if pt % 2 else nc.sync).dma_start(
                        lhs[:].rearrange("p (k j) -> p k j", k=NKT),
                        wn_d[:, pt * 128:(pt + 1) * 128]
                        .rearrange("(k fi) j -> fi k j", fi=128))
                at = apool.tile([128, QPC], BF16)
                ps0 = pspool.tile([128, 512], F32, tag="ps")
                ps1 = pspool.tile([128, 512], F32, tag="ps")
                lhs_r = lhs[:].rearrange("p (k j) -> p k j", k=NKT)
                # 4 DoubleRow matmuls contract k-pairs, a plain fp8 matmul
                # takes the 9th k-tile; each weight feeds both q chunks
                for kt in range(4):
                    w2 = lhs_r[:, 2 * kt:2 * kt + 2]
                    nc.tensor.matmul(ps0[:], w2, rhs_r[:, 2 * kt:2 * kt + 2,
                                                       0:512],
                                     start=(kt == 0), stop=False,
                                     perf_mode=DR)
                    nc.tensor.matmul(ps1[:], w2, rhs_r[:, 2 * kt:2 * kt + 2,
                                                       512:1024],
                                     start=(kt == 0), stop=False,
                                     perf_mode=DR)
                w = lhs[:, 8 * 128:9 * 128]
                nc.tensor.matmul(ps0[:], w, rhs_sb[:, 8 * QPC:8 * QPC + 512],
                                 start=False, stop=True)
                nc.tensor.matmul(ps1[:], w, rhs_sb[:, 8 * QPC + 512:
                                                   8 * QPC + 1024],
                                 start=False, stop=True)
                nc.scalar.activation(at[:, 0:512], ps0[:], AFT.Exp,
                                     bias=m_sb[:, pt:pt + 1],
                                     scale=SCALE / 16.0)
                nc.scalar.activation(at[:, 512:1024], ps1[:], AFT.Exp,
                                     bias=m_sb[:, pt:pt + 1],
                                     scale=SCALE / 16.0)
                nc.vector.tensor_add(acc[:], acc[:], at[:])
                # stagger xu prefetch 8 tiles behind to keep the early
                # queues clear for rhs/lhs
                if pt >= 6:
                    j = pt - 6
                    xt = xupool.tile([128, 1024], BF16, tag="xt",
                                     name=f"xt0_{j}")
                    (nc.sync, nc.gpsimd, nc.scalar)[j % 3].dma_start(
                        xt[:], xu_d[j * 128:(j + 1) * 128, 0:1024])
                    xts[(0, j)] = xt
                a_tiles.append(at)
            for j in range(NPT - 6, NPT):
                xt = xupool.tile([128, 1024], BF16, tag="xt",
                                 name=f"xt0_{j}")
                (nc.sync, nc.gpsimd, nc.scalar)[j % 3].dma_start(
                    xt[:], xu_d[j * 128:(j + 1) * 128, 0:1024])
                xts[(0, j)] = xt

            # ---- phase C: colT[q, ck] = sum_p A[p, q] xu[p, ck], scaled ---
            # (phase B colsum matmuls are slotted in after the first q-tile's
            #  matmuls so the PE never idles at the A->C boundary)
            first = True
            for ch in range(NCH):
                for qt in range(NQT):
                    psa = pspool.tile([128, 512], F32, tag="ps",
                                      name=f"psc{ch}_{qt}a")
                    psb = pspool.tile([128, 512], F32, tag="ps",
                                      name=f"psc{ch}_{qt}b")
                    for pt in range(NPT):
                        if (ch, pt) not in xts:
                            xt = xupool.tile([128, 1024], BF16, tag="xt",
                                             name=f"xt{ch}_{pt}")
                            (nc.gpsimd if pt % 2 else nc.sync).dma_start(
                                xt[:], xu_d[pt * 128:(pt + 1) * 128,
                                            ch * 1024:(ch + 1) * 1024])
                            xts[(ch, pt)] = xt
                        xt = xts[(ch, pt)]
                        w = a_tiles[pt][:, qt * 128:(qt + 1) * 128]
                        nc.tensor.matmul(psa[:], w, xt[:, 0:512],
                                         start=(pt == 0), stop=(pt == NPT - 1))
                        nc.tensor.matmul(psb[:], w, xt[:, 512:1024],
                                         start=(pt == 0), stop=(pt == NPT - 1))
                    if first:
                        # ---- phase B: r8[i, j] = 1/colsum(q=j*128+i) ------
                        first = False
                        cs_ps = pspool.tile([128, NQT], F32, tag="ps",
                                            name="csps")
                        for j in range(NQT):
                            nc.tensor.matmul(
                                cs_ps[:, j:j + 1],
                                acc[:, j * 128:(j + 1) * 128], ones_col[:],
                                start=True, stop=True)
                        nc.vector.tensor_copy(r8[:], cs_ps[:])
                        nc.vector.reciprocal(r8[:], r8[:])
                    ot = opool.tile([128, 1024], BF16)
                    if ch == NCH - 1 and qt == NQT - 1:
                        # final tile: 256-wide reads + stores fanned over
                        # three queues to cut the end-of-kernel drain tail
                        qs = (nc.scalar, nc.sync, nc.gpsimd, nc.scalar)
                        for si in range(4):
                            ps_ = (psa if si < 2 else psb)
                            nc.vector.tensor_scalar_mul(
                                ot[:, si * 256:(si + 1) * 256],
                                ps_[:, (si % 2) * 256:(si % 2) * 256 + 256],
                                r8[:, qt:qt + 1])
                            qs[si].dma_start(
                                col_d[qt * 128:(qt + 1) * 128,
                                      ch * 1024 + si * 256:
                                      ch * 1024 + (si + 1) * 256],
                                ot[:, si * 256:(si + 1) * 256])
                    else:
                        nc.vector.tensor_scalar_mul(ot[:, 0:512], psa[:],
                                                    r8[:, qt:qt + 1])
                        nc.vector.tensor_scalar_mul(ot[:, 512:1024], psb[:],
                                                    r8[:, qt:qt + 1])
                        nc.scalar.dma_start(
                            col_d[qt * 128:(qt + 1) * 128,
                                  ch * 1024:ch * 1024 + 512], ot[:, 0:512])
                        nc.gpsimd.dma_start(
                            col_d[qt * 128:(qt + 1) * 128,
                                  ch * 1024 + 512:(ch + 1) * 1024],
                            ot[:, 512:1024])
                # chunk done: its xu tiles recycle via the pool
                for pt in range(NPT):
                    del xts[(ch, pt)]
    nc.compile()
    return nc


def _host_prep(x, mask):
    """Per-batch GEMM-ready operands (kappa-major feature layout)."""
    out = []
    for b in range(B):
        xr = x[b, :, ::RATE, ::RATE]
        xrp = np.pad(xr, ((0, 0), (1, 1), (1, 1)))
        pr = np.empty((9, C, L), np.float32)
        for di in range(3):
            for dj in range(3):
                pr[di * 3 + dj] = xrp[:, di:di + Hr, dj:dj + Wr].reshape(C, L)
        pr = pr.reshape(F, L)
        denom = np.sqrt((pr * pr).sum(0, dtype=np.float64).astype(np.float32)
                        + np.float32(F * EPS))

        mr = mask[b, :, ::RATE, ::RATE]
        mrp = np.pad(mr, ((0, 0), (1, 1), (1, 1)))
        msum = np.zeros((1, L), np.float32)
        for di in range(3):
            for dj in range(3):
                msum += mrp[:, di:di + Hr, dj:dj + Wr].reshape(1, L)
        mfilt = (msum[0] == 0.0).astype(np.float32)

        wn = (pr / denom[None, :]) * mfilt[None, :]

        # global softmax shift: exact after colsum normalization as long as
        # exp stays in f32 range; diag scores are ~denom_q so the midpoint
        # keeps args within +-5*spread
        cshift = 0.5 * float(denom.max() + denom.min())

        xp = np.pad(x[b], ((0, 0), (1, 1), (1, 1)))
        xu = np.empty((L, 16, C), np.float32)
        for i in range(4):
            for j in range(4):
                blk = xp[:, i:i + 2 * Hr:2, j:j + 2 * Wr:2]
                xu[:, i * 4 + j, :] = blk.reshape(C, L).T
        out.append((np.ascontiguousarray((wn * WNS).astype(NPFP8)),
                    pr, cshift, mfilt,
                    np.ascontiguousarray(
                        xu.reshape(L, CK).astype(NPBF16))))
    return out


def _col2im(col):
    """col [L, CK] -> [C, H, W] overlap-add, /4."""
    canvas = np.zeros((C, H + 2, W + 2), np.float32)
    blk = col.reshape(Hr, Wr, 16, C)
    for i in range(4):
        for j in range(4):
            canvas[:, i:i + 2 * Hr:2, j:j + 2 * Wr:2] += \
                blk[:, :, i * 4 + j, :].transpose(2, 0, 1)
    return canvas[:, 1:1 + H, 1:1 + W] / 4.0


def kernel(x, mask):
    x = np.asarray(x, np.float32)
    mask = np.asarray(mask, np.float32)
    if "nc" not in _CACHE:
        _CACHE["nc"] = _build_nc()
    nc = _CACHE["nc"]

    prep = _host_prep(x, mask)
    in_maps = []
    for core in range(N_CORES):
        b, g = divmod(core, 4)
        wn, pr, cshift, mfilt, xu = prep[b]
        q0 = g * QPC
        in_maps.append({
            "wn": wn,
            "prq": np.ascontiguousarray(pr[:, q0:q0 + QPC].astype(NPFP8)),
            "xu": xu,
            "mrow": np.ascontiguousarray(
                ((mfilt - 1.0) * 1e4 - SCALE * cshift)
                .astype(np.float32).reshape(32, 128).T),
        })

    _CACHE["in_maps"] = in_maps
    res = run_bass_kernel_spmd(nc, in_maps, list(range(N_CORES)))

    out = np.empty((B, C, H, W), np.float32)
    for b in range(B):
        col = np.concatenate(
            [np.asarray(res.results[b * 4 + g]["col"], np.float32)
             for g in range(4)], axis=0)
        out[b] = _col2im(col)
    return out


# revision 28
# speedup vs baseline: 1.7729x; 1.0037x over previous
"""ContextualAttention TRN2 kernel.

Full inputs -> full output. Sharding: 8 cores = 2 batches x 4 q-blocks of the
L=4096 attention-column dimension. Each core computes, for its 1024 columns q:

  S[p, q]  = sum_f wn[f, p] * pr[f, q]          (QK^T, K = 1152 = 9 x 128)
  E[p, q]  = exp(10 * S - 10*c + m_p)           (c = global shift; any per-q
                                                 factor cancels in the colsum
                                                 normalization, so a single
                                                 constant that keeps exp in
                                                 f32 range is exact)
  A[p, q]  = E * mfilt_p                         (post-softmax patch mask)
  colT[q,:] = (A^T @ xu) / colsum_q              (conv_transpose as GEMM)

Host side: unfold / normalization prep (pure index shuffles + one divide) and
the final col2im overlap-add.  wn has the pre-softmax mask and 1/denom_p
folded in on the host.  All GEMM operands travel as bf16.

Schedule notes: every stationary (weight) tile feeds two 512-wide matmuls
back to back to amortize LDWEIGHTS; phase C walks q-tiles one at a time with
two PSUM banks each so bank recycling never stalls the PE; xu tiles are kept
resident across the 8 q-tile passes of each 1024-wide ck chunk and prefetched
on the scalar DMA queue.
"""
import numpy as np
import ml_dtypes

import concourse.bass as bass
import concourse.bacc as bacc
import concourse.mybir as mybir
from concourse import tile
from concourse.bass_utils import run_bass_kernel_spmd

F32 = mybir.dt.float32
BF16 = mybir.dt.bfloat16
FP8 = mybir.dt.float8e4
DR = mybir.MatmulPerfMode.DoubleRow
AFT = mybir.ActivationFunctionType

B, C, H, W = 2, 128, 128, 128
RATE, BS = 2, 3                # attention rate, block size
Hr, Wr = H // RATE, W // RATE  # 64, 64
L = Hr * Wr                    # 4096
F = C * BS * BS                # 1152 contraction dim, 9 k-tiles
CK = C * 16                    # 2048 deconv output cols (kappa*128 + c)
QPC = L // 4                   # 1024 q columns per core
EPS = 1e-4
SCALE = 10.0
N_CORES = 8
NPBF16 = ml_dtypes.bfloat16
NPFP8 = ml_dtypes.float8_e4m3
WNS = 16.0   # fp8 pre-scale on wn, undone by the activation scale

_CACHE = {}


def _build_nc():
    nc = bacc.Bacc(None)
    wn_d = nc.declare_dram_parameter("wn", [F, L], FP8, isOutput=False)
    prq_d = nc.declare_dram_parameter("prq", [F, QPC], FP8, isOutput=False)
    xu_d = nc.declare_dram_parameter("xu", [L, CK], BF16, isOutput=False)
    mrow_d = nc.declare_dram_parameter("mrow", [128, 32], F32, isOutput=False)
    # tile-contiguous output: row block (ch*NQT + qt)*128 holds the
    # [128, 1024] tile, so each store is one contiguous 256 KB burst
    col_d = nc.declare_dram_parameter("col", [2 * QPC, 1024], BF16,
                                      isOutput=True)

    NPT = L // 128    # 32 p tiles
    NKT = F // 128    # 9 k tiles
    NQT = QPC // 128  # 8 q tiles
    NCH = CK // 1024  # 2 ck chunks of 1024

    with tile.TileContext(nc) as tc:
        with (
            tc.tile_pool(name="apool", bufs=NPT) as apool,
            tc.tile_pool(name="const", bufs=1) as cpool,
            tc.tile_pool(name="rhs", bufs=1) as rhspool,
            tc.tile_pool(name="lhs", bufs=3) as lhspool,
            tc.tile_pool(name="xus", bufs=40) as xupool,
            tc.tile_pool(name="outs", bufs=2) as opool,
            tc.tile_pool(name="rows", bufs=2) as rowpool,
            tc.tile_pool(name="ps", bufs=8, space="PSUM") as pspool,
        ):
            # ---- resident loads -------------------------------------------
            # per-k chunks so the first matmul only waits for its own slice;
            # the k=0 chunk and the first lhs tile go first on the sync queue
            rhs_sb = rhspool.tile([128, NKT * QPC], FP8)        # 9 KB/part
            rhs_r = rhs_sb[:].rearrange("p (k q) -> p k q", k=NKT)
            prq_r = prq_d[:].rearrange("(k fi) q -> fi k q", fi=128)
            lhs0 = lhspool.tile([128, NKT * 128], FP8, tag="lhs")
            nc.gpsimd.dma_start(
                lhs0[:].rearrange("p (k j) -> p k j", k=NKT),
                wn_d[:, 0:128].rearrange("(k fi) j -> fi k j", fi=128))
            for k in range(NKT):
                (nc.sync, nc.scalar, nc.gpsimd)[k % 3].dma_start(
                    rhs_r[:, k], prq_r[:, k])
            m_sb = cpool.tile([128, 32], F32)
            nc.gpsimd.dma_start(m_sb[:], mrow_d[:])
            ones_col = cpool.tile([128, 1], F32)
            nc.gpsimd.memset(ones_col[:], 1.0)
            acc = cpool.tile([128, QPC], F32)
            nc.gpsimd.memset(acc[:], 0.0)
            r8 = cpool.tile([128, NQT], F32)

            # ---- phase A: S = wn^T @ prq, E = exp(10 S + bias), acc += E --
            # each k weight tile feeds both 512-wide q chunks back to back;
            # xu's first ck chunk prefetches on the vector queue meanwhile
            xts = {}
            a_tiles = []
            for pt in range(NPT):
                if pt == 0:
                    lhs = lhs0
                else:
                    lhs = lhspool.tile([128, NKT * 128], FP8, tag="lhs")
                    (nc.gpsimd if pt % 2 else nc.sync).dma_start(
                        lhs[:].rearrange("p (k j) -> p k j", k=NKT),
                        wn_d[:, pt * 128:(pt + 1) * 128]
                        .rearrange("(k fi) j -> fi k j", fi=128))
                at = apool.tile([128, QPC], BF16)
                ps0 = pspool.tile([128, 512], F32, tag="ps")
                ps1 = pspool.tile([128, 512], F32, tag="ps")
                lhs_r = lhs[:].rearrange("p (k j) -> p k j", k=NKT)
                # 4 DoubleRow matmuls contract k-pairs, a plain fp8 matmul
                # takes the 9th k-tile; each weight feeds both q chunks
                for kt in range(4):
                    w2 = lhs_r[:, 2 * kt:2 * kt + 2]
                    nc.tensor.matmul(ps0[:], w2, rhs_r[:, 2 * kt:2 * kt + 2,
                                                       0:512],
                                     start=(kt == 0), stop=False,
                                     perf_mode=DR)
                    nc.tensor.matmul(ps1[:], w2, rhs_r[:, 2 * kt:2 * kt + 2,
                                                       512:1024],
                                     start=(kt == 0), stop=False,
                                     perf_mode=DR)
                w = lhs[:, 8 * 128:9 * 128]
                nc.tensor.matmul(ps0[:], w, rhs_sb[:, 8 * QPC:8 * QPC + 512],
                                 start=False, stop=True)
                nc.tensor.matmul(ps1[:], w, rhs_sb[:, 8 * QPC + 512:
                                                   8 * QPC + 1024],
                                 start=False, stop=True)
                nc.scalar.activation(at[:, 0:512], ps0[:], AFT.Exp,
                                     bias=m_sb[:, pt:pt + 1],
                                     scale=SCALE / 16.0)
                nc.scalar.activation(at[:, 512:1024], ps1[:], AFT.Exp,
                                     bias=m_sb[:, pt:pt + 1],
                                     scale=SCALE / 16.0)
                nc.vector.tensor_add(acc[:], acc[:], at[:])
                # stagger xu prefetch 8 tiles behind to keep the early
                # queues clear for rhs/lhs
                if pt >= 8:
                    j = pt - 8
                    xt = xupool.tile([128, 1024], BF16, tag="xt",
                                     name=f"xt0_{j}")
                    (nc.sync, nc.gpsimd, nc.scalar)[j % 3].dma_start(
                        xt[:], xu_d[j * 128:(j + 1) * 128, 0:1024])
                    xts[(0, j)] = xt
                a_tiles.append(at)
            for j in range(NPT - 8, NPT):
                xt = xupool.tile([128, 1024], BF16, tag="xt",
                                 name=f"xt0_{j}")
                (nc.sync, nc.gpsimd, nc.scalar)[j % 3].dma_start(
                    xt[:], xu_d[j * 128:(j + 1) * 128, 0:1024])
                xts[(0, j)] = xt

            # ---- phase C: colT[q, ck] = sum_p A[p, q] xu[p, ck], scaled ---
            # (phase B colsum matmuls are slotted in after the first q-tile's
            #  matmuls so the PE never idles at the A->C boundary)
            first = True
            for ch in range(NCH):
                for qt in range(NQT):
                    psa = pspool.tile([128, 512], F32, tag="ps",
                                      name=f"psc{ch}_{qt}a")
                    psb = pspool.tile([128, 512], F32, tag="ps",
                                      name=f"psc{ch}_{qt}b")
                    for pt in range(NPT):
                        if (ch, pt) not in xts:
                            xt = xupool.tile([128, 1024], BF16, tag="xt",
                                             name=f"xt{ch}_{pt}")
                            (nc.gpsimd if pt % 2 else nc.sync).dma_start(
                                xt[:], xu_d[pt * 128:(pt + 1) * 128,
                                            ch * 1024:(ch + 1) * 1024])
                            xts[(ch, pt)] = xt
                        xt = xts[(ch, pt)]
                        w = a_tiles[pt][:, qt * 128:(qt + 1) * 128]
                        nc.tensor.matmul(psa[:], w, xt[:, 0:512],
                                         start=(pt == 0), stop=(pt == NPT - 1))
                        nc.tensor.matmul(psb[:], w, xt[:, 512:1024],
                                         start=(pt == 0), stop=(pt == NPT - 1))
                    if first:
                        # ---- phase B: r8[i, j] = 1/colsum(q=j*128+i) ------
                        first = False
                        cs_ps = pspool.tile([128, NQT], F32, tag="ps",
                                            name="csps")
                        for j in range(NQT):
                            nc.tensor.matmul(
                                cs_ps[:, j:j + 1],
                                acc[:, j * 128:(j + 1) * 128], ones_col[:],
                                start=True, stop=True)
                        nc.vector.tensor_copy(r8[:], cs_ps[:])
                        nc.vector.reciprocal(r8[:], r8[:])
                    ot = opool.tile([128, 1024], BF16)
                    nc.vector.tensor_scalar_mul(ot[:, 0:512], psa[:],
                                                r8[:, qt:qt + 1])
                    nc.vector.tensor_scalar_mul(ot[:, 512:1024], psb[:],
                                                r8[:, qt:qt + 1])
                    r0 = (ch * NQT + qt) * 128
                    if ch == NCH - 1 and qt == NQT - 1:
                        # final tile: contiguous quarter-row stores fanned
                        # over three queues to cut the drain tail
                        qs = (nc.scalar, nc.sync, nc.gpsimd, nc.scalar)
                        for si in range(4):
                            qs[si].dma_start(
                                col_d[r0 + si * 32:r0 + (si + 1) * 32, :],
                                ot[si * 32:(si + 1) * 32, :])
                    else:
                        nc.scalar.dma_start(col_d[r0:r0 + 64, :],
                                            ot[0:64, :])
                        nc.gpsimd.dma_start(col_d[r0 + 64:r0 + 128, :],
                                            ot[64:128, :])
                # chunk done: its xu tiles recycle via the pool
                for pt in range(NPT):
                    del xts[(ch, pt)]
    nc.compile()
    return nc


def _host_prep(x, mask):
    """Per-batch GEMM-ready operands (kappa-major feature layout)."""
    out = []
    for b in range(B):
        xr = x[b, :, ::RATE, ::RATE]
        xrp = np.pad(xr, ((0, 0), (1, 1), (1, 1)))
        pr = np.empty((9, C, L), np.float32)
        for di in range(3):
            for dj in range(3):
                pr[di * 3 + dj] = xrp[:, di:di + Hr, dj:dj + Wr].reshape(C, L)
        pr = pr.reshape(F, L)
        denom = np.sqrt((pr * pr).sum(0, dtype=np.float64).astype(np.float32)
                        + np.float32(F * EPS))

        mr = mask[b, :, ::RATE, ::RATE]
        mrp = np.pad(mr, ((0, 0), (1, 1), (1, 1)))
        msum = np.zeros((1, L), np.float32)
        for di in range(3):
            for dj in range(3):
                msum += mrp[:, di:di + Hr, dj:dj + Wr].reshape(1, L)
        mfilt = (msum[0] == 0.0).astype(np.float32)

        wn = (pr / denom[None, :]) * mfilt[None, :]

        # global softmax shift: exact after colsum normalization as long as
        # exp stays in f32 range; diag scores are ~denom_q so the midpoint
        # keeps args within +-5*spread
        cshift = 0.5 * float(denom.max() + denom.min())

        xp = np.pad(x[b], ((0, 0), (1, 1), (1, 1)))
        xu = np.empty((L, 16, C), np.float32)
        for i in range(4):
            for j in range(4):
                blk = xp[:, i:i + 2 * Hr:2, j:j + 2 * Wr:2]
                xu[:, i * 4 + j, :] = blk.reshape(C, L).T
        out.append((np.ascontiguousarray((wn * WNS).astype(NPFP8)),
                    pr, cshift, mfilt,
                    np.ascontiguousarray(
                        xu.reshape(L, CK).astype(NPBF16))))
    return out


def _col2im(col):
    """col [L, CK] -> [C, H, W] overlap-add, /4."""
    canvas = np.zeros((C, H + 2, W + 2), np.float32)
    blk = col.reshape(Hr, Wr, 16, C)
    for i in range(4):
        for j in range(4):
            canvas[:, i:i + 2 * Hr:2, j:j + 2 * Wr:2] += \
                blk[:, :, i * 4 + j, :].transpose(2, 0, 1)
    return canvas[:, 1:1 + H, 1:1 + W] / 4.0


def kernel(x, mask):
    x = np.asarray(x, np.float32)
    mask = np.asarray(mask, np.float32)
    if "nc" not in _CACHE:
        _CACHE["nc"] = _build_nc()
    nc = _CACHE["nc"]

    prep = _host_prep(x, mask)
    in_maps = []
    for core in range(N_CORES):
        b, g = divmod(core, 4)
        wn, pr, cshift, mfilt, xu = prep[b]
        q0 = g * QPC
        in_maps.append({
            "wn": wn,
            "prq": np.ascontiguousarray(pr[:, q0:q0 + QPC].astype(NPFP8)),
            "xu": xu,
            "mrow": np.ascontiguousarray(
                ((mfilt - 1.0) * 1e4 - SCALE * cshift)
                .astype(np.float32).reshape(32, 128).T),
        })

    _CACHE["in_maps"] = in_maps
    res = run_bass_kernel_spmd(nc, in_maps, list(range(N_CORES)))

    out = np.empty((B, C, H, W), np.float32)
    for b in range(B):
        col = np.concatenate(
            [np.asarray(res.results[b * 4 + g]["col"], np.float32)
             .reshape(2, QPC, 1024).transpose(1, 0, 2).reshape(QPC, CK)
             for g in range(4)], axis=0)
        out[b] = _col2im(col)
    return out


# revision 31
# speedup vs baseline: 1.7913x; 1.0104x over previous
"""ContextualAttention TRN2 kernel.

Full inputs -> full output. Sharding: 8 cores = 2 batches x 4 q-blocks of the
L=4096 attention-column dimension. Each core computes, for its 1024 columns q:

  S[p, q]  = sum_f wn[f, p] * pr[f, q]          (QK^T, K = 1152 = 9 x 128)
  E[p, q]  = exp(10 * S - 10*c + m_p)           (c = global shift; any per-q
                                                 factor cancels in the colsum
                                                 normalization, so a single
                                                 constant that keeps exp in
                                                 f32 range is exact)
  A[p, q]  = E * mfilt_p                         (post-softmax patch mask)
  colT[q,:] = (A^T @ xu) / colsum_q              (conv_transpose as GEMM)

Host side: unfold / normalization prep (pure index shuffles + one divide) and
the final col2im overlap-add.  wn has the pre-softmax mask and 1/denom_p
folded in on the host.  All GEMM operands travel as bf16.

Schedule notes: every stationary (weight) tile feeds two 512-wide matmuls
back to back to amortize LDWEIGHTS; phase C walks q-tiles one at a time with
two PSUM banks each so bank recycling never stalls the PE; xu tiles are kept
resident across the 8 q-tile passes of each 1024-wide ck chunk and prefetched
on the scalar DMA queue.
"""
import numpy as np
import ml_dtypes

import concourse.bass as bass
import concourse.bacc as bacc
import concourse.mybir as mybir
from concourse import tile
from concourse.bass_utils import run_bass_kernel_spmd

F32 = mybir.dt.float32
BF16 = mybir.dt.bfloat16
FP8 = mybir.dt.float8e4
DR = mybir.MatmulPerfMode.DoubleRow
AFT = mybir.ActivationFunctionType

B, C, H, W = 2, 128, 128, 128
RATE, BS = 2, 3                # attention rate, block size
Hr, Wr = H // RATE, W // RATE  # 64, 64
L = Hr * Wr                    # 4096
F = C * BS * BS                # 1152 contraction dim, 9 k-tiles
CK = C * 16                    # 2048 deconv output cols (kappa*128 + c)
QPC = L // 4                   # 1024 q columns per core
EPS = 1e-4
SCALE = 10.0
N_CORES = 8
NPBF16 = ml_dtypes.bfloat16
NPFP8 = ml_dtypes.float8_e4m3
WNS = 16.0   # fp8 pre-scale on wn, undone by the activation scale

_CACHE = {}


def _build_nc():
    nc = bacc.Bacc(None)
    wn_d = nc.declare_dram_parameter("wn", [F, L], FP8, isOutput=False)
    prq_d = nc.declare_dram_parameter("prq", [F, QPC], FP8, isOutput=False)
    xu_d = nc.declare_dram_parameter("xu", [L, CK], BF16, isOutput=False)
    mrow_d = nc.declare_dram_parameter("mrow", [128, 32], F32, isOutput=False)
    # tile-contiguous output: row block (ch*NQT + qt)*128 holds the
    # [128, 1024] tile, so each store is one contiguous 256 KB burst
    col_d = nc.declare_dram_parameter("col", [2 * QPC, 1024], BF16,
                                      isOutput=True)

    NPT = L // 128    # 32 p tiles
    NKT = F // 128    # 9 k tiles
    NQT = QPC // 128  # 8 q tiles
    NCH = CK // 1024  # 2 ck chunks of 1024

    with tile.TileContext(nc) as tc:
        with (
            tc.tile_pool(name="apool", bufs=NPT) as apool,
            tc.tile_pool(name="const", bufs=1) as cpool,
            tc.tile_pool(name="rhs", bufs=1) as rhspool,
            tc.tile_pool(name="lhs", bufs=6) as lhspool,
            tc.tile_pool(name="xus", bufs=40) as xupool,
            tc.tile_pool(name="outs", bufs=2) as opool,
            tc.tile_pool(name="rows", bufs=2) as rowpool,
            tc.tile_pool(name="ps", bufs=8, space="PSUM") as pspool,
        ):
            # ---- resident loads -------------------------------------------
            # per-k chunks so the first matmul only waits for its own slice;
            # the k=0 chunk and the first lhs tile go first on the sync queue
            rhs_sb = rhspool.tile([128, NKT * QPC], FP8)        # 9 KB/part
            rhs_r = rhs_sb[:].rearrange("p (k q) -> p k q", k=NKT)
            prq_r = prq_d[:].rearrange("(k fi) q -> fi k q", fi=128)
            lhs0 = lhspool.tile([128, NKT * 128], FP8, tag="lhs")
            nc.gpsimd.dma_start(
                lhs0[:].rearrange("p (k j) -> p k j", k=NKT),
                wn_d[:, 0:128].rearrange("(k fi) j -> fi k j", fi=128))
            for k in range(NKT):
                (nc.sync, nc.scalar, nc.gpsimd)[k % 3].dma_start(
                    rhs_r[:, k], prq_r[:, k])
            m_sb = cpool.tile([128, 32], F32)
            nc.gpsimd.dma_start(m_sb[:], mrow_d[:])
            ones_col = cpool.tile([128, 1], F32)
            nc.gpsimd.memset(ones_col[:], 1.0)
            acc = cpool.tile([128, QPC], F32)
            nc.gpsimd.memset(acc[:], 0.0)
            r8 = cpool.tile([128, NQT], F32)

            # ---- phase A: S = wn^T @ prq, E = exp(10 S + bias), acc += E --
            # each k weight tile feeds both 512-wide q chunks back to back;
            # xu's first ck chunk prefetches on the vector queue meanwhile
            xts = {}
            a_tiles = []
            lhs_tiles = {0: lhs0}

            def load_lhs(i):
                t = lhspool.tile([128, NKT * 128], FP8, tag="lhs")
                (nc.gpsimd if i % 2 else nc.sync).dma_start(
                    t[:].rearrange("p (k j) -> p k j", k=NKT),
                    wn_d[:, i * 128:(i + 1) * 128]
                    .rearrange("(k fi) j -> fi k j", fi=128))
                lhs_tiles[i] = t

            for i in range(1, 5):
                load_lhs(i)
            for pt in range(NPT):
                if pt + 5 < NPT:
                    load_lhs(pt + 5)
                lhs = lhs_tiles.pop(pt)
                at = apool.tile([128, QPC], BF16)
                ps0 = pspool.tile([128, 512], F32, tag="ps")
                ps1 = pspool.tile([128, 512], F32, tag="ps")
                lhs_r = lhs[:].rearrange("p (k j) -> p k j", k=NKT)
                # 4 DoubleRow matmuls contract k-pairs, a plain fp8 matmul
                # takes the 9th k-tile; each weight feeds both q chunks
                for kt in range(4):
                    w2 = lhs_r[:, 2 * kt:2 * kt + 2]
                    nc.tensor.matmul(ps0[:], w2, rhs_r[:, 2 * kt:2 * kt + 2,
                                                       0:512],
                                     start=(kt == 0), stop=False,
                                     perf_mode=DR)
                    nc.tensor.matmul(ps1[:], w2, rhs_r[:, 2 * kt:2 * kt + 2,
                                                       512:1024],
                                     start=(kt == 0), stop=False,
                                     perf_mode=DR)
                w = lhs[:, 8 * 128:9 * 128]
                nc.tensor.matmul(ps0[:], w, rhs_sb[:, 8 * QPC:8 * QPC + 512],
                                 start=False, stop=True)
                nc.tensor.matmul(ps1[:], w, rhs_sb[:, 8 * QPC + 512:
                                                   8 * QPC + 1024],
                                 start=False, stop=True)
                nc.scalar.activation(at[:, 0:512], ps0[:], AFT.Exp,
                                     bias=m_sb[:, pt:pt + 1],
                                     scale=SCALE / 16.0)
                nc.scalar.activation(at[:, 512:1024], ps1[:], AFT.Exp,
                                     bias=m_sb[:, pt:pt + 1],
                                     scale=SCALE / 16.0)
                nc.vector.tensor_add(acc[:], acc[:], at[:])
                # stagger xu prefetch 8 tiles behind to keep the early
                # queues clear for rhs/lhs
                if pt >= 8:
                    j = pt - 8
                    xt = xupool.tile([128, 1024], BF16, tag="xt",
                                     name=f"xt0_{j}")
                    (nc.sync, nc.gpsimd, nc.scalar)[j % 3].dma_start(
                        xt[:], xu_d[j * 128:(j + 1) * 128, 0:1024])
                    xts[(0, j)] = xt
                a_tiles.append(at)
            for j in range(NPT - 8, NPT):
                xt = xupool.tile([128, 1024], BF16, tag="xt",
                                 name=f"xt0_{j}")
                (nc.sync, nc.gpsimd, nc.scalar)[j % 3].dma_start(
                    xt[:], xu_d[j * 128:(j + 1) * 128, 0:1024])
                xts[(0, j)] = xt

            # ---- phase C: colT[q, ck] = sum_p A[p, q] xu[p, ck], scaled ---
            # (phase B colsum matmuls are slotted in after the first q-tile's
            #  matmuls so the PE never idles at the A->C boundary)
            first = True
            for ch in range(NCH):
                for qt in range(NQT):
                    psa = pspool.tile([128, 512], F32, tag="ps",
                                      name=f"psc{ch}_{qt}a")
                    psb = pspool.tile([128, 512], F32, tag="ps",
                                      name=f"psc{ch}_{qt}b")
                    for pt in range(NPT):
                        if (ch, pt) not in xts:
                            xt = xupool.tile([128, 1024], BF16, tag="xt",
                                             name=f"xt{ch}_{pt}")
                            (nc.gpsimd if pt % 2 else nc.sync).dma_start(
                                xt[:], xu_d[pt * 128:(pt + 1) * 128,
                                            ch * 1024:(ch + 1) * 1024])
                            xts[(ch, pt)] = xt
                        xt = xts[(ch, pt)]
                        w = a_tiles[pt][:, qt * 128:(qt + 1) * 128]
                        nc.tensor.matmul(psa[:], w, xt[:, 0:512],
                                         start=(pt == 0), stop=(pt == NPT - 1))
                        nc.tensor.matmul(psb[:], w, xt[:, 512:1024],
                                         start=(pt == 0), stop=(pt == NPT - 1))
                    if first:
                        # ---- phase B: r8[i, j] = 1/colsum(q=j*128+i) ------
                        first = False
                        cs_ps = pspool.tile([128, NQT], F32, tag="ps",
                                            name="csps")
                        for j in range(NQT):
                            nc.tensor.matmul(
                                cs_ps[:, j:j + 1],
                                acc[:, j * 128:(j + 1) * 128], ones_col[:],
                                start=True, stop=True)
                        nc.vector.tensor_copy(r8[:], cs_ps[:])
                        nc.vector.reciprocal(r8[:], r8[:])
                    ot = opool.tile([128, 1024], BF16)
                    nc.vector.tensor_scalar_mul(ot[:, 0:512], psa[:],
                                                r8[:, qt:qt + 1])
                    nc.vector.tensor_scalar_mul(ot[:, 512:1024], psb[:],
                                                r8[:, qt:qt + 1])
                    r0 = (ch * NQT + qt) * 128
                    if ch == NCH - 1 and qt == NQT - 1:
                        # final tile: contiguous row-slice stores fanned
                        # evenly over three queues to cut the drain tail
                        qs = (nc.scalar, nc.sync, nc.gpsimd)
                        bounds = (0, 21, 42, 64, 85, 106, 128)
                        for si in range(6):
                            a, b = bounds[si], bounds[si + 1]
                            qs[si % 3].dma_start(
                                col_d[r0 + a:r0 + b, :], ot[a:b, :])
                    else:
                        nc.scalar.dma_start(col_d[r0:r0 + 64, :],
                                            ot[0:64, :])
                        nc.gpsimd.dma_start(col_d[r0 + 64:r0 + 128, :],
                                            ot[64:128, :])
                # chunk done: its xu tiles recycle via the pool
                for pt in range(NPT):
                    del xts[(ch, pt)]
    nc.compile()
    return nc


def _host_prep(x, mask):
    """Per-batch GEMM-ready operands (kappa-major feature layout)."""
    out = []
    for b in range(B):
        xr = x[b, :, ::RATE, ::RATE]
        xrp = np.pad(xr, ((0, 0), (1, 1), (1, 1)))
        pr = np.empty((9, C, L), np.float32)
        for di in range(3):
            for dj in range(3):
                pr[di * 3 + dj] = xrp[:, di:di + Hr, dj:dj + Wr].reshape(C, L)
        pr = pr.reshape(F, L)
        denom = np.sqrt((pr * pr).sum(0, dtype=np.float64).astype(np.float32)
                        + np.float32(F * EPS))

        mr = mask[b, :, ::RATE, ::RATE]
        mrp = np.pad(mr, ((0, 0), (1, 1), (1, 1)))
        msum = np.zeros((1, L), np.float32)
        for di in range(3):
            for dj in range(3):
                msum += mrp[:, di:di + Hr, dj:dj + Wr].reshape(1, L)
        mfilt = (msum[0] == 0.0).astype(np.float32)

        wn = (pr / denom[None, :]) * mfilt[None, :]

        # global softmax shift: exact after colsum normalization as long as
        # exp stays in f32 range; diag scores are ~denom_q so the midpoint
        # keeps args within +-5*spread
        cshift = 0.5 * float(denom.max() + denom.min())

        xp = np.pad(x[b], ((0, 0), (1, 1), (1, 1)))
        xu = np.empty((L, 16, C), np.float32)
        for i in range(4):
            for j in range(4):
                blk = xp[:, i:i + 2 * Hr:2, j:j + 2 * Wr:2]
                xu[:, i * 4 + j, :] = blk.reshape(C, L).T
        out.append((np.ascontiguousarray((wn * WNS).astype(NPFP8)),
                    pr, cshift, mfilt,
                    np.ascontiguousarray(
                        xu.reshape(L, CK).astype(NPBF16))))
    return out


def _col2im(col):
    """col [L, CK] -> [C, H, W] overlap-add, /4."""
    canvas = np.zeros((C, H + 2, W + 2), np.float32)
    blk = col.reshape(Hr, Wr, 16, C)
    for i in range(4):
        for j in range(4):
            canvas[:, i:i + 2 * Hr:2, j:j + 2 * Wr:2] += \
                blk[:, :, i * 4 + j, :].transpose(2, 0, 1)
    return canvas[:, 1:1 + H, 1:1 + W] / 4.0


def kernel(x, mask):
    x = np.asarray(x, np.float32)
    mask = np.asarray(mask, np.float32)
    if "nc" not in _CACHE:
        _CACHE["nc"] = _build_nc()
    nc = _CACHE["nc"]

    prep = _host_prep(x, mask)
    in_maps = []
    for core in range(N_CORES):
        b, g = divmod(core, 4)
        wn, pr, cshift, mfilt, xu = prep[b]
        q0 = g * QPC
        in_maps.append({
            "wn": wn,
            "prq": np.ascontiguousarray(pr[:, q0:q0 + QPC].astype(NPFP8)),
            "xu": xu,
            "mrow": np.ascontiguousarray(
                ((mfilt - 1.0) * 1e4 - SCALE * cshift)
                .astype(np.float32).reshape(32, 128).T),
        })

    _CACHE["in_maps"] = in_maps
    res = run_bass_kernel_spmd(nc, in_maps, list(range(N_CORES)))

    out = np.empty((B, C, H, W), np.float32)
    for b in range(B):
        col = np.concatenate(
            [np.asarray(res.results[b * 4 + g]["col"], np.float32)
             .reshape(2, QPC, 1024).transpose(1, 0, 2).reshape(QPC, CK)
             for g in range(4)], axis=0)
        out[b] = _col2im(col)
    return out


# revision 33
# speedup vs baseline: 1.8190x; 1.0155x over previous
"""ContextualAttention TRN2 kernel.

Full inputs -> full output. Sharding: 8 cores = 2 batches x 4 q-blocks of the
L=4096 attention-column dimension. Each core computes, for its 1024 columns q:

  S[p, q]  = sum_f wn[f, p] * pr[f, q]          (QK^T, K = 1152 = 9 x 128)
  E[p, q]  = exp(10 * S - 10*c + m_p)           (c = global shift; any per-q
                                                 factor cancels in the colsum
                                                 normalization, so a single
                                                 constant that keeps exp in
                                                 f32 range is exact)
  A[p, q]  = E * mfilt_p                         (post-softmax patch mask)
  colT[q,:] = (A^T @ xu) / colsum_q              (conv_transpose as GEMM)

Host side: unfold / normalization prep (pure index shuffles + one divide) and
the final col2im overlap-add.  wn has the pre-softmax mask and 1/denom_p
folded in on the host.  All GEMM operands travel as bf16.

Schedule notes: every stationary (weight) tile feeds two 512-wide matmuls
back to back to amortize LDWEIGHTS; phase C walks q-tiles one at a time with
two PSUM banks each so bank recycling never stalls the PE; xu tiles are kept
resident across the 8 q-tile passes of each 1024-wide ck chunk and prefetched
on the scalar DMA queue.
"""
import numpy as np
import ml_dtypes

import concourse.bass as bass
import concourse.bacc as bacc
import concourse.mybir as mybir
from concourse import tile
from concourse.bass_utils import run_bass_kernel_spmd

F32 = mybir.dt.float32
BF16 = mybir.dt.bfloat16
FP8 = mybir.dt.float8e4
DR = mybir.MatmulPerfMode.DoubleRow
AFT = mybir.ActivationFunctionType

B, C, H, W = 2, 128, 128, 128
RATE, BS = 2, 3                # attention rate, block size
Hr, Wr = H // RATE, W // RATE  # 64, 64
L = Hr * Wr                    # 4096
F = C * BS * BS                # 1152 contraction dim, 9 k-tiles
CK = C * 16                    # 2048 deconv output cols (kappa*128 + c)
QPC = L // 4                   # 1024 q columns per core
EPS = 1e-4
SCALE = 10.0
N_CORES = 8
NPBF16 = ml_dtypes.bfloat16
NPFP8 = ml_dtypes.float8_e4m3
WNS = 16.0   # fp8 pre-scale on wn, undone by the activation scale

_CACHE = {}


def _build_nc():
    nc = bacc.Bacc(None)
    wn_d = nc.declare_dram_parameter("wn", [F, L], FP8, isOutput=False)
    prq_d = nc.declare_dram_parameter("prq", [F, QPC], FP8, isOutput=False)
    xu_d = nc.declare_dram_parameter("xu", [L, CK], BF16, isOutput=False)
    mrow_d = nc.declare_dram_parameter("mrow", [128, 32], F32, isOutput=False)
    # tile-contiguous output: row block (ch*NQT + qt)*128 holds the
    # [128, 1024] tile, so each store is one contiguous 256 KB burst
    col_d = nc.declare_dram_parameter("col", [2 * QPC, 1024], BF16,
                                      isOutput=True)

    NPT = L // 128    # 32 p tiles
    NKT = F // 128    # 9 k tiles
    NQT = QPC // 128  # 8 q tiles
    NCH = CK // 1024  # 2 ck chunks of 1024

    with tile.TileContext(nc) as tc:
        with (
            tc.tile_pool(name="apool", bufs=NPT) as apool,
            tc.tile_pool(name="const", bufs=1) as cpool,
            tc.tile_pool(name="rhs", bufs=1) as rhspool,
            tc.tile_pool(name="lhs", bufs=6) as lhspool,
            tc.tile_pool(name="xus", bufs=40) as xupool,
            tc.tile_pool(name="outs", bufs=2) as opool,
            tc.tile_pool(name="rows", bufs=2) as rowpool,
            tc.tile_pool(name="ps", bufs=8, space="PSUM") as pspool,
        ):
            # ---- resident loads -------------------------------------------
            # per-k chunks so the first matmul only waits for its own slice;
            # the k=0 chunk and the first lhs tile go first on the sync queue
            rhs_sb = rhspool.tile([128, NKT * QPC], FP8)        # 9 KB/part
            rhs_r = rhs_sb[:].rearrange("p (k q) -> p k q", k=NKT)
            prq_r = prq_d[:].rearrange("(k fi) q -> fi k q", fi=128)
            lhs0 = lhspool.tile([128, NKT * 128], FP8, tag="lhs")
            nc.gpsimd.dma_start(
                lhs0[:].rearrange("p (k j) -> p k j", k=NKT),
                wn_d[:, 0:128].rearrange("(k fi) j -> fi k j", fi=128))
            for k in range(NKT):
                (nc.sync, nc.scalar, nc.gpsimd)[k % 3].dma_start(
                    rhs_r[:, k], prq_r[:, k])
            m_sb = cpool.tile([128, 32], F32)
            nc.gpsimd.dma_start(m_sb[:], mrow_d[:])
            ones_col = cpool.tile([128, 1], F32)
            nc.gpsimd.memset(ones_col[:], 1.0)
            acc = cpool.tile([128, QPC], F32)
            nc.gpsimd.memset(acc[:], 0.0)
            r8 = cpool.tile([128, NQT], F32)

            # ---- phase A: S = wn^T @ prq, E = exp(10 S + bias), acc += E --
            # each k weight tile feeds both 512-wide q chunks back to back;
            # xu's first ck chunk prefetches on the vector queue meanwhile
            xts = {}
            a_tiles = []
            lhs_tiles = {0: lhs0}

            def load_lhs(i):
                t = lhspool.tile([128, NKT * 128], FP8, tag="lhs")
                (nc.gpsimd if i % 2 else nc.sync).dma_start(
                    t[:].rearrange("p (k j) -> p k j", k=NKT),
                    wn_d[:, i * 128:(i + 1) * 128]
                    .rearrange("(k fi) j -> fi k j", fi=128))
                lhs_tiles[i] = t

            for i in range(1, 5):
                load_lhs(i)
            for pt in range(NPT):
                if pt + 5 < NPT:
                    load_lhs(pt + 5)
                lhs = lhs_tiles.pop(pt)
                at = apool.tile([128, QPC], BF16)
                ps0 = pspool.tile([128, 512], F32, tag="ps")
                ps1 = pspool.tile([128, 512], F32, tag="ps")
                lhs_r = lhs[:].rearrange("p (k j) -> p k j", k=NKT)
                # 4 DoubleRow matmuls contract k-pairs, a plain fp8 matmul
                # takes the 9th k-tile; each weight feeds both q chunks
                for kt in range(4):
                    w2 = lhs_r[:, 2 * kt:2 * kt + 2]
                    nc.tensor.matmul(ps0[:], w2, rhs_r[:, 2 * kt:2 * kt + 2,
                                                       0:512],
                                     start=(kt == 0), stop=False,
                                     perf_mode=DR)
                    nc.tensor.matmul(ps1[:], w2, rhs_r[:, 2 * kt:2 * kt + 2,
                                                       512:1024],
                                     start=(kt == 0), stop=False,
                                     perf_mode=DR)
                w = lhs[:, 8 * 128:9 * 128]
                nc.tensor.matmul(ps0[:], w, rhs_sb[:, 8 * QPC:8 * QPC + 512],
                                 start=False, stop=True)
                nc.tensor.matmul(ps1[:], w, rhs_sb[:, 8 * QPC + 512:
                                                   8 * QPC + 1024],
                                 start=False, stop=True)
                nc.scalar.activation(at[:, 0:512], ps0[:], AFT.Exp,
                                     bias=m_sb[:, pt:pt + 1],
                                     scale=SCALE / 16.0)
                nc.scalar.activation(at[:, 512:1024], ps1[:], AFT.Exp,
                                     bias=m_sb[:, pt:pt + 1],
                                     scale=SCALE / 16.0)
                nc.vector.tensor_add(acc[:], acc[:], at[:])
                # stagger xu prefetch 8 tiles behind to keep the early
                # queues clear for rhs/lhs
                if pt >= 8:
                    j = pt - 8
                    xt = xupool.tile([128, 1024], BF16, tag="xt",
                                     name=f"xt0_{j}")
                    (nc.sync, nc.gpsimd, nc.scalar)[j % 3].dma_start(
                        xt[:], xu_d[j * 128:(j + 1) * 128, 0:1024])
                    xts[(0, j)] = xt
                a_tiles.append(at)
            for j in range(NPT - 8, NPT):
                xt = xupool.tile([128, 1024], BF16, tag="xt",
                                 name=f"xt0_{j}")
                (nc.sync, nc.gpsimd, nc.scalar)[j % 3].dma_start(
                    xt[:], xu_d[j * 128:(j + 1) * 128, 0:1024])
                xts[(0, j)] = xt

            # ---- phase C: colT[q, ck] = sum_p A[p, q] xu[p, ck], scaled ---
            # (phase B colsum matmuls are slotted in after the first q-tile's
            #  matmuls so the PE never idles at the A->C boundary)
            first = True
            for ch in range(NCH):
                for qt in range(NQT):
                    psa = pspool.tile([128, 512], F32, tag="ps",
                                      name=f"psc{ch}_{qt}a")
                    psb = pspool.tile([128, 512], F32, tag="ps",
                                      name=f"psc{ch}_{qt}b")
                    for pt in range(NPT):
                        if (ch, pt) not in xts:
                            xt = xupool.tile([128, 1024], BF16, tag="xt",
                                             name=f"xt{ch}_{pt}")
                            (nc.gpsimd if pt % 2 else nc.sync).dma_start(
                                xt[:], xu_d[pt * 128:(pt + 1) * 128,
                                            ch * 1024:(ch + 1) * 1024])
                            xts[(ch, pt)] = xt
                        xt = xts[(ch, pt)]
                        w = a_tiles[pt][:, qt * 128:(qt + 1) * 128]
                        nc.tensor.matmul(psa[:], w, xt[:, 0:512],
                                         start=(pt == 0), stop=(pt == NPT - 1))
                        nc.tensor.matmul(psb[:], w, xt[:, 512:1024],
                                         start=(pt == 0), stop=(pt == NPT - 1))
                    if ch == 0 and qt == NQT - 1:
                        # slots freed by now: prefetch next chunk's head
                        for j in range(8):
                            xt = xupool.tile([128, 1024], BF16, tag="xt",
                                             name=f"xt1_{j}")
                            (nc.sync if j % 2 else nc.gpsimd).dma_start(
                                xt[:], xu_d[j * 128:(j + 1) * 128,
                                            1024:2048])
                            xts[(1, j)] = xt
                    if first:
                        # ---- phase B: r8[i, j] = 1/colsum(q=j*128+i) ------
                        first = False
                        cs_ps = pspool.tile([128, NQT], F32, tag="ps",
                                            name="csps")
                        for j in range(NQT):
                            nc.tensor.matmul(
                                cs_ps[:, j:j + 1],
                                acc[:, j * 128:(j + 1) * 128], ones_col[:],
                                start=True, stop=True)
                        nc.vector.tensor_copy(r8[:], cs_ps[:])
                        nc.vector.reciprocal(r8[:], r8[:])
                    ot = opool.tile([128, 1024], BF16)
                    nc.vector.tensor_scalar_mul(ot[:, 0:512], psa[:],
                                                r8[:, qt:qt + 1])
                    nc.vector.tensor_scalar_mul(ot[:, 512:1024], psb[:],
                                                r8[:, qt:qt + 1])
                    r0 = (ch * NQT + qt) * 128
                    if ch == NCH - 1 and qt == NQT - 1:
                        # final tile: contiguous row-slice stores fanned
                        # evenly over three queues to cut the drain tail
                        qs = (nc.scalar, nc.sync, nc.gpsimd)
                        bounds = (0, 21, 42, 64, 85, 106, 128)
                        for si in range(6):
                            a, b = bounds[si], bounds[si + 1]
                            qs[si % 3].dma_start(
                                col_d[r0 + a:r0 + b, :], ot[a:b, :])
                    else:
                        # rotate store queues so no single queue builds a
                        # backlog toward the end of the kernel
                        t = ch * NQT + qt
                        qa, qb = ((nc.scalar, nc.gpsimd),
                                  (nc.sync, nc.scalar),
                                  (nc.gpsimd, nc.sync))[t % 3]
                        qa.dma_start(col_d[r0:r0 + 64, :], ot[0:64, :])
                        qb.dma_start(col_d[r0 + 64:r0 + 128, :],
                                     ot[64:128, :])
                # chunk done: its xu tiles recycle via the pool
                for pt in range(NPT):
                    del xts[(ch, pt)]
    nc.compile()
    return nc


def _host_prep(x, mask):
    """Per-batch GEMM-ready operands (kappa-major feature layout)."""
    out = []
    for b in range(B):
        xr = x[b, :, ::RATE, ::RATE]
        xrp = np.pad(xr, ((0, 0), (1, 1), (1, 1)))
        pr = np.empty((9, C, L), np.float32)
        for di in range(3):
            for dj in range(3):
                pr[di * 3 + dj] = xrp[:, di:di + Hr, dj:dj + Wr].reshape(C, L)
        pr = pr.reshape(F, L)
        denom = np.sqrt((pr * pr).sum(0, dtype=np.float64).astype(np.float32)
                        + np.float32(F * EPS))

        mr = mask[b, :, ::RATE, ::RATE]
        mrp = np.pad(mr, ((0, 0), (1, 1), (1, 1)))
        msum = np.zeros((1, L), np.float32)
        for di in range(3):
            for dj in range(3):
                msum += mrp[:, di:di + Hr, dj:dj + Wr].reshape(1, L)
        mfilt = (msum[0] == 0.0).astype(np.float32)

        wn = (pr / denom[None, :]) * mfilt[None, :]

        # global softmax shift: exact after colsum normalization as long as
        # exp stays in f32 range; diag scores are ~denom_q so the midpoint
        # keeps args within +-5*spread
        cshift = 0.5 * float(denom.max() + denom.min())

        xp = np.pad(x[b], ((0, 0), (1, 1), (1, 1)))
        xu = np.empty((L, 16, C), np.float32)
        for i in range(4):
            for j in range(4):
                blk = xp[:, i:i + 2 * Hr:2, j:j + 2 * Wr:2]
                xu[:, i * 4 + j, :] = blk.reshape(C, L).T
        out.append((np.ascontiguousarray((wn * WNS).astype(NPFP8)),
                    pr, cshift, mfilt,
                    np.ascontiguousarray(
                        xu.reshape(L, CK).astype(NPBF16))))
    return out


def _col2im(col):
    """col [L, CK] -> [C, H, W] overlap-add, /4."""
    canvas = np.zeros((C, H + 2, W + 2), np.float32)
    blk = col.reshape(Hr, Wr, 16, C)
    for i in range(4):
        for j in range(4):
            canvas[:, i:i + 2 * Hr:2, j:j + 2 * Wr:2] += \
                blk[:, :, i * 4 + j, :].transpose(2, 0, 1)
    return canvas[:, 1:1 + H, 1:1 + W] / 4.0


def kernel(x, mask):
    x = np.asarray(x, np.float32)
    mask = np.asarray(mask, np.float32)
    if "nc" not in _CACHE:
        _CACHE["nc"] = _build_nc()
    nc = _CACHE["nc"]

    prep = _host_prep(x, mask)
    in_maps = []
    for core in range(N_CORES):
        b, g = divmod(core, 4)
        wn, pr, cshift, mfilt, xu = prep[b]
        q0 = g * QPC
        in_maps.append({
            "wn": wn,
            "prq": np.ascontiguousarray(pr[:, q0:q0 + QPC].astype(NPFP8)),
            "xu": xu,
            "mrow": np.ascontiguousarray(
                ((mfilt - 1.0) * 1e4 - SCALE * cshift)
                .astype(np.float32).reshape(32, 128).T),
        })

    _CACHE["in_maps"] = in_maps
    res = run_bass_kernel_spmd(nc, in_maps, list(range(N_CORES)))

    out = np.empty((B, C, H, W), np.float32)
    for b in range(B):
        col = np.concatenate(
            [np.asarray(res.results[b * 4 + g]["col"], np.float32)
             .reshape(2, QPC, 1024).transpose(1, 0, 2).reshape(QPC, CK)
             for g in range(4)], axis=0)
        out[b] = _col2im(col)
    return out


# revision 36
# speedup vs baseline: 1.8229x; 1.0022x over previous
"""ContextualAttention TRN2 kernel.

Full inputs -> full output. Sharding: 8 cores = 2 batches x 4 q-blocks of the
L=4096 attention-column dimension. Each core computes, for its 1024 columns q:

  S[p, q]  = sum_f wn[f, p] * pr[f, q]          (QK^T, K = 1152 = 9 x 128)
  E[p, q]  = exp(10 * S - 10*c + m_p)           (c = global shift; any per-q
                                                 factor cancels in the colsum
                                                 normalization, so a single
                                                 constant that keeps exp in
                                                 f32 range is exact)
  A[p, q]  = E * mfilt_p                         (post-softmax patch mask)
  colT[q,:] = (A^T @ xu) / colsum_q              (conv_transpose as GEMM)

Host side: unfold / normalization prep (pure index shuffles + one divide) and
the final col2im overlap-add.  wn has the pre-softmax mask and 1/denom_p
folded in on the host.  All GEMM operands travel as bf16.

Schedule notes: every stationary (weight) tile feeds two 512-wide matmuls
back to back to amortize LDWEIGHTS; phase C walks q-tiles one at a time with
two PSUM banks each so bank recycling never stalls the PE; xu tiles are kept
resident across the 8 q-tile passes of each 1024-wide ck chunk and prefetched
on the scalar DMA queue.
"""
import numpy as np
import ml_dtypes

import concourse.bass as bass
import concourse.bacc as bacc
import concourse.mybir as mybir
from concourse import tile
from concourse.bass_utils import run_bass_kernel_spmd

F32 = mybir.dt.float32
BF16 = mybir.dt.bfloat16
FP8 = mybir.dt.float8e4
DR = mybir.MatmulPerfMode.DoubleRow
AFT = mybir.ActivationFunctionType

B, C, H, W = 2, 128, 128, 128
RATE, BS = 2, 3                # attention rate, block size
Hr, Wr = H // RATE, W // RATE  # 64, 64
L = Hr * Wr                    # 4096
F = C * BS * BS                # 1152 contraction dim, 9 k-tiles
CK = C * 16                    # 2048 deconv output cols (kappa*128 + c)
QPC = L // 4                   # 1024 q columns per core
EPS = 1e-4
SCALE = 10.0
N_CORES = 8
NPBF16 = ml_dtypes.bfloat16
NPFP8 = ml_dtypes.float8_e4m3
WNS = 16.0   # fp8 pre-scale on wn, undone by the activation scale

_CACHE = {}


def _build_nc():
    nc = bacc.Bacc(None)
    wn_d = nc.declare_dram_parameter("wn", [F, L], FP8, isOutput=False)
    prq_d = nc.declare_dram_parameter("prq", [F, QPC], FP8, isOutput=False)
    xu_d = nc.declare_dram_parameter("xu", [L, CK], BF16, isOutput=False)
    mrow_d = nc.declare_dram_parameter("mrow", [128, 32], F32, isOutput=False)
    # tile-contiguous output: row block (ch*NQT + qt)*128 holds the
    # [128, 1024] tile, so each store is one contiguous 256 KB burst
    col_d = nc.declare_dram_parameter("col", [2 * QPC, 1024], BF16,
                                      isOutput=True)

    NPT = L // 128    # 32 p tiles
    NKT = F // 128    # 9 k tiles
    NQT = QPC // 128  # 8 q tiles
    NCH = CK // 1024  # 2 ck chunks of 1024

    with tile.TileContext(nc) as tc:
        with (
            tc.tile_pool(name="apool", bufs=NPT) as apool,
            tc.tile_pool(name="const", bufs=1) as cpool,
            tc.tile_pool(name="rhs", bufs=1) as rhspool,
            tc.tile_pool(name="lhs", bufs=6) as lhspool,
            tc.tile_pool(name="xus", bufs=40) as xupool,
            tc.tile_pool(name="outs", bufs=2) as opool,
            tc.tile_pool(name="rows", bufs=2) as rowpool,
            tc.tile_pool(name="ps", bufs=8, space="PSUM") as pspool,
        ):
            # ---- resident loads -------------------------------------------
            # per-k chunks so the first matmul only waits for its own slice;
            # the k=0 chunk and the first lhs tile go first on the sync queue
            rhs_sb = rhspool.tile([128, NKT * QPC], FP8)        # 9 KB/part
            rhs_r = rhs_sb[:].rearrange("p (k q) -> p k q", k=NKT)
            prq_r = prq_d[:].rearrange("(k fi) q -> fi k q", fi=128)
            lhs0 = lhspool.tile([128, NKT * 128], FP8, tag="lhs")
            nc.gpsimd.dma_start(
                lhs0[:].rearrange("p (k j) -> p k j", k=NKT),
                wn_d[:, 0:128].rearrange("(k fi) j -> fi k j", fi=128))
            for k in range(NKT):
                (nc.sync, nc.scalar, nc.gpsimd)[k % 3].dma_start(
                    rhs_r[:, k], prq_r[:, k])
            m_sb = cpool.tile([128, 32], F32)
            nc.gpsimd.dma_start(m_sb[:], mrow_d[:])
            ones_col = cpool.tile([128, 1], F32)
            nc.gpsimd.memset(ones_col[:], 1.0)
            acc = cpool.tile([128, QPC], F32)
            nc.gpsimd.memset(acc[:], 0.0)
            r8 = cpool.tile([128, NQT], F32)

            # ---- phase A: S = wn^T @ prq, E = exp(10 S + bias), acc += E --
            # each k weight tile feeds both 512-wide q chunks back to back;
            # xu's first ck chunk prefetches on the vector queue meanwhile
            xts = {}
            a_tiles = []
            lhs_tiles = {0: lhs0}

            def load_lhs(i):
                t = lhspool.tile([128, NKT * 128], FP8, tag="lhs")
                (nc.gpsimd if i % 2 else nc.sync).dma_start(
                    t[:].rearrange("p (k j) -> p k j", k=NKT),
                    wn_d[:, i * 128:(i + 1) * 128]
                    .rearrange("(k fi) j -> fi k j", fi=128))
                lhs_tiles[i] = t

            for i in range(1, 5):
                load_lhs(i)
            for pt in range(NPT):
                if pt + 5 < NPT:
                    load_lhs(pt + 5)
                lhs = lhs_tiles.pop(pt)
                at = apool.tile([128, QPC], BF16)
                ps0 = pspool.tile([128, 512], F32, tag="ps")
                ps1 = pspool.tile([128, 512], F32, tag="ps")
                lhs_r = lhs[:].rearrange("p (k j) -> p k j", k=NKT)
                # plain fp8 matmul on the 9th k-tile first (depends on a
                # single rhs chunk, so the group starts ASAP at kernel
                # start), then 4 DoubleRow matmuls contract the k-pairs;
                # each weight feeds both q chunks
                w = lhs[:, 8 * 128:9 * 128]
                nc.tensor.matmul(ps0[:], w, rhs_sb[:, 8 * QPC:8 * QPC + 512],
                                 start=True, stop=False)
                nc.tensor.matmul(ps1[:], w, rhs_sb[:, 8 * QPC + 512:
                                                   8 * QPC + 1024],
                                 start=True, stop=False)
                for kt in range(4):
                    w2 = lhs_r[:, 2 * kt:2 * kt + 2]
                    nc.tensor.matmul(ps0[:], w2, rhs_r[:, 2 * kt:2 * kt + 2,
                                                       0:512],
                                     start=False, stop=(kt == 3),
                                     perf_mode=DR)
                    nc.tensor.matmul(ps1[:], w2, rhs_r[:, 2 * kt:2 * kt + 2,
                                                       512:1024],
                                     start=False, stop=(kt == 3),
                                     perf_mode=DR)
                nc.scalar.activation(at[:, 0:512], ps0[:], AFT.Exp,
                                     bias=m_sb[:, pt:pt + 1],
                                     scale=SCALE / 16.0)
                nc.scalar.activation(at[:, 512:1024], ps1[:], AFT.Exp,
                                     bias=m_sb[:, pt:pt + 1],
                                     scale=SCALE / 16.0)
                nc.vector.tensor_add(acc[:], acc[:], at[:])
                # stagger xu prefetch 8 tiles behind to keep the early
                # queues clear for rhs/lhs
                if pt >= 8:
                    j = pt - 8
                    xt = xupool.tile([128, 1024], BF16, tag="xt",
                                     name=f"xt0_{j}")
                    (nc.sync, nc.gpsimd, nc.scalar)[j % 3].dma_start(
                        xt[:], xu_d[j * 128:(j + 1) * 128, 0:1024])
                    xts[(0, j)] = xt
                a_tiles.append(at)
            for j in range(NPT - 8, NPT):
                xt = xupool.tile([128, 1024], BF16, tag="xt",
                                 name=f"xt0_{j}")
                (nc.sync, nc.gpsimd, nc.scalar)[j % 3].dma_start(
                    xt[:], xu_d[j * 128:(j + 1) * 128, 0:1024])
                xts[(0, j)] = xt

            # ---- phase C: colT[q, ck] = sum_p A[p, q] xu[p, ck], scaled ---
            # (phase B colsum matmuls are slotted in after the first q-tile's
            #  matmuls so the PE never idles at the A->C boundary)
            first = True
            for ch in range(NCH):
                for qt in range(NQT):
                    psa = pspool.tile([128, 512], F32, tag="ps",
                                      name=f"psc{ch}_{qt}a")
                    psb = pspool.tile([128, 512], F32, tag="ps",
                                      name=f"psc{ch}_{qt}b")
                    for pt in range(NPT):
                        if (ch, pt) not in xts:
                            xt = xupool.tile([128, 1024], BF16, tag="xt",
                                             name=f"xt{ch}_{pt}")
                            (nc.gpsimd if pt % 2 else nc.sync).dma_start(
                                xt[:], xu_d[pt * 128:(pt + 1) * 128,
                                            ch * 1024:(ch + 1) * 1024])
                            xts[(ch, pt)] = xt
                        xt = xts[(ch, pt)]
                        w = a_tiles[pt][:, qt * 128:(qt + 1) * 128]
                        nc.tensor.matmul(psa[:], w, xt[:, 0:512],
                                         start=(pt == 0), stop=(pt == NPT - 1))
                        nc.tensor.matmul(psb[:], w, xt[:, 512:1024],
                                         start=(pt == 0), stop=(pt == NPT - 1))
                    if ch == 0 and qt == NQT - 1:
                        # slots freed by now: prefetch next chunk's head
                        for j in range(8):
                            xt = xupool.tile([128, 1024], BF16, tag="xt",
                                             name=f"xt1_{j}")
                            (nc.sync if j % 2 else nc.gpsimd).dma_start(
                                xt[:], xu_d[j * 128:(j + 1) * 128,
                                            1024:2048])
                            xts[(1, j)] = xt
                    if first:
                        # ---- phase B: r8[i, j] = 1/colsum(q=j*128+i) ------
                        first = False
                        cs_ps = pspool.tile([128, NQT], F32, tag="ps",
                                            name="csps")
                        for j in range(NQT):
                            nc.tensor.matmul(
                                cs_ps[:, j:j + 1],
                                acc[:, j * 128:(j + 1) * 128], ones_col[:],
                                start=True, stop=True)
                        nc.vector.tensor_copy(r8[:], cs_ps[:])
                        nc.vector.reciprocal(r8[:], r8[:])
                    ot = opool.tile([128, 1024], BF16)
                    r0 = (ch * NQT + qt) * 128
                    if ch == NCH - 1 and qt == NQT - 1:
                        # final tile: 256-wide reads interleaved with
                        # contiguous row-slice stores over three queues so
                        # streaming starts right after the last matmul
                        for si in range(4):
                            ps_ = (psa if si < 2 else psb)
                            nc.vector.tensor_scalar_mul(
                                ot[:, si * 256:(si + 1) * 256],
                                ps_[:, (si % 2) * 256:(si % 2) * 256 + 256],
                                r8[:, qt:qt + 1])
                        qs = (nc.scalar, nc.sync, nc.gpsimd)
                        bounds = (0, 21, 42, 64, 85, 106, 128)
                        for si in range(6):
                            a, b = bounds[si], bounds[si + 1]
                            qs[si % 3].dma_start(
                                col_d[r0 + a:r0 + b, :], ot[a:b, :])
                    else:
                        nc.vector.tensor_scalar_mul(ot[:, 0:512], psa[:],
                                                    r8[:, qt:qt + 1])
                        nc.vector.tensor_scalar_mul(ot[:, 512:1024], psb[:],
                                                    r8[:, qt:qt + 1])
                        # rotate store queues so no single queue builds a
                        # backlog toward the end of the kernel
                        t = ch * NQT + qt
                        qa, qb = ((nc.scalar, nc.gpsimd),
                                  (nc.sync, nc.scalar),
                                  (nc.gpsimd, nc.sync))[t % 3]
                        qa.dma_start(col_d[r0:r0 + 64, :], ot[0:64, :])
                        qb.dma_start(col_d[r0 + 64:r0 + 128, :],
                                     ot[64:128, :])
                # chunk done: its xu tiles recycle via the pool
                for pt in range(NPT):
                    del xts[(ch, pt)]
    nc.compile()
    return nc


def _host_prep(x, mask):
    """Per-batch GEMM-ready operands (kappa-major feature layout)."""
    out = []
    for b in range(B):
        xr = x[b, :, ::RATE, ::RATE]
        xrp = np.pad(xr, ((0, 0), (1, 1), (1, 1)))
        pr = np.empty((9, C, L), np.float32)
        for di in range(3):
            for dj in range(3):
                pr[di * 3 + dj] = xrp[:, di:di + Hr, dj:dj + Wr].reshape(C, L)
        pr = pr.reshape(F, L)
        denom = np.sqrt((pr * pr).sum(0, dtype=np.float64).astype(np.float32)
                        + np.float32(F * EPS))

        mr = mask[b, :, ::RATE, ::RATE]
        mrp = np.pad(mr, ((0, 0), (1, 1), (1, 1)))
        msum = np.zeros((1, L), np.float32)
        for di in range(3):
            for dj in range(3):
                msum += mrp[:, di:di + Hr, dj:dj + Wr].reshape(1, L)
        mfilt = (msum[0] == 0.0).astype(np.float32)

        wn = (pr / denom[None, :]) * mfilt[None, :]

        # global softmax shift: exact after colsum normalization as long as
        # exp stays in f32 range; diag scores are ~denom_q so the midpoint
        # keeps args within +-5*spread
        cshift = 0.5 * float(denom.max() + denom.min())

        xp = np.pad(x[b], ((0, 0), (1, 1), (1, 1)))
        xu = np.empty((L, 16, C), np.float32)
        for i in range(4):
            for j in range(4):
                blk = xp[:, i:i + 2 * Hr:2, j:j + 2 * Wr:2]
                xu[:, i * 4 + j, :] = blk.reshape(C, L).T
        out.append((np.ascontiguousarray((wn * WNS).astype(NPFP8)),
                    pr, cshift, mfilt,
                    np.ascontiguousarray(
                        xu.reshape(L, CK).astype(NPBF16))))
    return out


def _col2im(col):
    """col [L, CK] -> [C, H, W] overlap-add, /4."""
    canvas = np.zeros((C, H + 2, W + 2), np.float32)
    blk = col.reshape(Hr, Wr, 16, C)
    for i in range(4):
        for j in range(4):
            canvas[:, i:i + 2 * Hr:2, j:j + 2 * Wr:2] += \
                blk[:, :, i * 4 + j, :].transpose(2, 0, 1)
    return canvas[:, 1:1 + H, 1:1 + W] / 4.0


def kernel(x, mask):
    x = np.asarray(x, np.float32)
    mask = np.asarray(mask, np.float32)
    if "nc" not in _CACHE:
        _CACHE["nc"] = _build_nc()
    nc = _CACHE["nc"]

    prep = _host_prep(x, mask)
    in_maps = []
    for core in range(N_CORES):
        b, g = divmod(core, 4)
        wn, pr, cshift, mfilt, xu = prep[b]
        q0 = g * QPC
        in_maps.append({
            "wn": wn,
            "prq": np.ascontiguousarray(pr[:, q0:q0 + QPC].astype(NPFP8)),
            "xu": xu,
            "mrow": np.ascontiguousarray(
                ((mfilt - 1.0) * 1e4 - SCALE * cshift)
                .astype(np.float32).reshape(32, 128).T),
        })

    _CACHE["in_maps"] = in_maps
    res = run_bass_kernel_spmd(nc, in_maps, list(range(N_CORES)))

    out = np.empty((B, C, H, W), np.float32)
    for b in range(B):
        col = np.concatenate(
            [np.asarray(res.results[b * 4 + g]["col"], np.float32)
             .reshape(2, QPC, 1024).transpose(1, 0, 2).reshape(QPC, CK)
             for g in range(4)], axis=0)
        out[b] = _col2im(col)
    return out
